# revision 1
# baseline (speedup 1.0000x reference)
"""Sharded attention kernel for Trainium2 (8 NeuronCores, Bass/Tile).

Module: x->(wq,wk,wv) qk-norm + rope + GQA self-attn  (+)  gated cross-attn
over y->(wk_y,wv_y), then wo.  B=2, S=2048, D=2048, H=16, KV=8, HD=128,
YL=256, YD=1024.

Sharding: 2-way batch DP x 4-way head TP.  Core c handles batch c//4 and
head group g=c%4 (q heads 4g..4g+3, kv heads 2g..2g+1, y-heads
(4g%8)..(4g%8)+3).  wo is row-sharded; the 4 partial outputs per batch are
summed on the host.  The q/k/ky layernorms normalize over the *full* flat
head dim, so each core computes partial (sum, sumsq) stats and three small
in-kernel AllReduces (groups [[0..3],[4..7]]) produce the full-row moments;
each collective's latency is hidden under independent projection work.

Everything on device is fp16 (fp32 PSUM accumulation and fp32 LN moments),
which measured ~4x more accurate than bf16 at identical PE throughput.
Layouts are transposed ([dim, seq]) so projections and QK^T need no
on-device transposes.  RoPE's even/odd pairing is handled by permuting
wq/wk/wk_y columns to [evens|odds] per head (invariant for q.k dots) plus a
128x128 half-swap matmul.  Scores are computed transposed (keys on
partitions) so the key mask folds into the exp() bias as a per-partition
column and P feeds PV with no transpose; softmax skips max-subtraction
(post-LN scores are O(1); masked keys get -1e30 -> exp 0).  Softmax
denominators accumulate on DVE in fp16 (per-partition rounding cancels in
the final 128-way ones-matmul reduction).
"""
import sys

sys.path.insert(0, "/opt/trn_rl_repo")

import numpy as np
import ml_dtypes

import concourse.bass as bass  # noqa: F401
import concourse.tile as tile
from concourse import bacc, mybir
from concourse import bass_utils
from concourse.masks import make_identity

BF16 = mybir.dt.bfloat16
DT16 = mybir.dt.float16
F32 = mybir.dt.float32
NPBF16 = ml_dtypes.bfloat16
NP16 = np.float16

B, S, D, H, KV, YL, YD, HD = 2, 2048, 2048, 16, 8, 256, 1024, 128
N_CORES, TP = 8, 4
HPC, KVPC, YHPC = 4, 2, 4          # q / kv / y heads per core
QW, KW, YW = HPC * HD, KVPC * HD, YHPC * HD   # 512, 256, 512 output cols
NDC, NYC = D // 128, YD // 128     # contraction chunks: 16, 8
NSB, SB = 4, 512                   # seq blocks for projections
NQB, QB = 2, 1024                  # query blocks for attention
NKC = S // 128                     # 16 key chunks (self)
NYKC = YL // 128                   # 2 key chunks (cross)
NST = S // 128                     # 16 seq tiles for wo
EPS_QK, EPS_KY = 1e-5, 1e-6
NEG = -1.0e30

_RUNNER = None
_EXEC = None


def _build_program(use_cc=True):
    nc = bacc.Bacc("TRN2", target_bir_lowering=False, debug=False,
                   num_devices=N_CORES if use_cc else 1)

    def din(name, shape, dt=DT16):
        return nc.dram_tensor(name, shape, dt, kind="ExternalInput")

    t = dict(
        xT=din("xT", [D, S]),
        yT=din("yT", [YD, YL]),
        wq=din("wq", [D, QW]),
        wk=din("wk", [D, KW]),
        wv=din("wv", [D, KW]),
        wky=din("wky", [YD, YW]),
        wvy=din("wvy", [YD, YW]),
        wo=din("wo", [QW, D]),
        CC=din("CC", [128, S]),
        SSp=din("SSp", [128, S]),
        swapP=din("swapP", [128, 128]),
        qgc=din("qgc", [128, HPC], F32),
        kgc=din("kgc", [128, KVPC], F32),
        kygc=din("kygc", [128, YHPC], F32),
        qb=din("qb", [128, HPC], F32),
        kb=din("kb", [128, KVPC], F32),
        kyb=din("kyb", [128, YHPC], F32),
        xmask=din("xmask", [128, NKC], F32),
        ymask=din("ymask", [128, NYKC], F32),
        tg=din("tg", [1, YHPC], F32),
        out=nc.dram_tensor("out", [S, D], F32, kind="ExternalOutput"),
        kin=nc.dram_tensor("kin", [2, S], F32),
        kout=nc.dram_tensor("kout", [2, S], F32),
        kyin=nc.dram_tensor("kyin", [2, YL], F32),
        kyout=nc.dram_tensor("kyout", [2, YL], F32),
        qin=nc.dram_tensor("qin", [2, S], F32),
        qout=nc.dram_tensor("qout", [2, S], F32),
        lnr=nc.dram_tensor("lnr", [6, S], DT16),
        groups=[[0, 1, 2, 3], [4, 5, 6, 7]],
        use_cc=use_cc,
    )

    with tile.TileContext(nc) as tc:
        _emit(nc, tc, t)
    nc.compile()
    return nc


def _emit(nc, tc, t):
    AF = mybir.ActivationFunctionType
    Alu = mybir.AluOpType

    def strip_load(dst, src_ap, nchunk, strips=4, q=None):
        step = nchunk // strips
        eng = q or nc.sync
        for s0 in range(0, nchunk, step):
            eng.dma_start(dst[:, s0:s0 + step, :],
                          src_ap[:, s0:s0 + step, :])

    cm_consts = tc.tile_pool(name="consts", bufs=1)
    consts = cm_consts.__enter__()

    # ---------------- constants / small inputs ----------------
    ident = consts.tile([128, 128], DT16, tag="ident", name="ident")
    make_identity(nc, ident[:, :])
    ones_col = consts.tile([128, 1], DT16, tag="ones_col", name="ones_col")
    nc.vector.memset(ones_col[:, :], 1.0)
    ones_row = consts.tile([1, 128], DT16, tag="ones_row", name="ones_row")
    nc.vector.memset(ones_row[:, :], 1.0)
    ones_bf = consts.tile([128, 1], BF16, tag="ones_bf", name="ones_bf")
    nc.vector.memset(ones_bf[:, :], 1.0)
    swp = consts.tile([128, 128], DT16, tag="swp", name="swp")
    cc = consts.tile([128, S], DT16, tag="cc", name="cc")
    ssp = consts.tile([128, S], DT16, tag="ssp", name="ssp")
    qg_sb = consts.tile([128, HPC], F32, tag="qgc", name="qgc")
    nc.gpsimd.dma_start(qg_sb[:, :], t["qgc"].ap())
    kg_sb = consts.tile([128, KVPC], F32, tag="kgc", name="kgc")
    nc.gpsimd.dma_start(kg_sb[:, :], t["kgc"].ap())
    kyg_sb = consts.tile([128, YHPC], F32, tag="kygc", name="kygc")
    nc.gpsimd.dma_start(kyg_sb[:, :], t["kygc"].ap())
    qb_sb = consts.tile([128, HPC], F32, tag="qb", name="qb")
    nc.gpsimd.dma_start(qb_sb[:, :], t["qb"].ap())
    kb_sb = consts.tile([128, KVPC], F32, tag="kb", name="kb")
    nc.gpsimd.dma_start(kb_sb[:, :], t["kb"].ap())
    kyb_sb = consts.tile([128, YHPC], F32, tag="kyb", name="kyb")
    nc.gpsimd.dma_start(kyb_sb[:, :], t["kyb"].ap())
    xm_sb = consts.tile([128, NKC], F32, tag="xm", name="xm")
    nc.gpsimd.dma_start(xm_sb[:, :], t["xmask"].ap())
    ym_sb = consts.tile([128, NYKC], F32, tag="ym", name="ym")
    nc.gpsimd.dma_start(ym_sb[:, :], t["ymask"].ap())
    tg_sb = consts.tile([1, YHPC], F32, tag="tg", name="tg")
    nc.gpsimd.dma_start(tg_sb[:, :], t["tg"].ap())
    zrow = consts.tile([1, S], F32, tag="zrow", name="zrow")
    nc.vector.memset(zrow[:, :], 0.0)

    # ---------------- phase-1 pools ----------------
    cm_raw = tc.tile_pool(name="p_raw", bufs=1)
    p_raw = cm_raw.__enter__()
    cm_ph1 = tc.tile_pool(name="p_ph1", bufs=1)
    p_ph1 = cm_ph1.__enter__()
    cm_x = tc.tile_pool(name="p_x", bufs=2)
    p_x = cm_x.__enter__()
    cm_wsq = tc.tile_pool(name="w_sq", bufs=2)
    w_sq = cm_wsq.__enter__()
    cm_stg = tc.tile_pool(name="w_stg", bufs=2)
    w_stg = cm_stg.__enter__()

    def stat_out(dram_row, ps_ap, blk):
        stg = w_stg.tile([1, SB], F32, tag="stg", name="stg")
        nc.scalar.activation(stg[:, :blk], ps_ap, AF.Copy)
        nc.sync.dma_start(dram_row, stg[:, :blk])

    wk_sb = p_ph1.tile([128, NDC, KW], DT16, tag="wk", name="wk")
    nc.sync.dma_start(wk_sb[:, :, :],
                      t["wk"].ap().rearrange("(c p) m -> p c m", p=128))
    wv_sb = p_ph1.tile([128, NDC, KW], DT16, tag="wv", name="wv")
    nc.sync.dma_start(wv_sb[:, :, :],
                      t["wv"].ap().rearrange("(c p) m -> p c m", p=128))
    xT_r0 = t["xT"].ap().rearrange("(c p) s -> p c s", p=128)
    wq_sb = p_ph1.tile([128, NDC, QW], DT16, tag="wq", name="wq")
    xtb0 = p_x.tile([128, NDC, SB], DT16, tag="xtb", name="xtb")
    wq_r = t["wq"].ap().rearrange("(c p) m -> p c m", p=128)
    xr0 = xT_r0[:, :, 0:SB]
    for s0 in range(0, NDC, 2):
        nc.sync.dma_start(wq_sb[:, s0:s0 + 2, :], wq_r[:, s0:s0 + 2, :])
        nc.sync.dma_start(xtb0[:, s0:s0 + 2, :], xr0[:, s0:s0 + 2, :])
    yt = p_ph1.tile([128, NYC, YL], DT16, tag="yt", name="yt")
    nc.sync.dma_start(yt[:, :, :],
                      t["yT"].ap().rearrange("(c p) s -> p c s", p=128))
    wky_sb = p_ph1.tile([128, NYC, YW], DT16, tag="wky", name="wky")
    nc.sync.dma_start(wky_sb[:, :, :],
                      t["wky"].ap().rearrange("(c p) m -> p c m", p=128))
    wvy_sb = p_ph1.tile([128, NYC, YW], DT16, tag="wvy", name="wvy")
    nc.sync.dma_start(wvy_sb[:, :, :],
                      t["wvy"].ap().rearrange("(c p) m -> p c m", p=128))

    kraw = [p_raw.tile([128, S], DT16, tag=f"kraw{i}", name=f"kraw{i}")
            for i in range(KVPC)]
    vraw = [p_raw.tile([128, S], DT16, tag=f"vraw{i}", name=f"vraw{i}")
            for i in range(KVPC)]
    qraw = [p_raw.tile([128, S], DT16, tag=f"qraw{i}", name=f"qraw{i}")
            for i in range(HPC)]
    ykraw = [p_raw.tile([128, YL], DT16, tag=f"ykraw{i}", name=f"ykraw{i}")
             for i in range(YHPC)]
    yvraw = [p_raw.tile([128, YL], DT16, tag=f"yvraw{i}", name=f"yvraw{i}")
             for i in range(YHPC)]

    cm_psA = tc.tile_pool(name="pp_projA", bufs=2, space="PSUM")
    cm_psB = tc.tile_pool(name="pp_statB", bufs=2, space="PSUM")
    pp_proj = cm_psA.__enter__()
    pp_stat = cm_psB.__enter__()

    xT_r = t["xT"].ap().rearrange("(c p) s -> p c s", p=128)

    # ============ phase 1a: q projections + q stats + AR-q ============
    def proj_block(w_sb, nchunk, src, col0, dst, sb, blk,
                   sum_ps=None, sq_ps=None, first=False, last=False):
        ps = pp_proj.tile([128, SB], F32, tag="proj", name="proj")
        for c in range(nchunk):
            nc.tensor.matmul(ps[:, :blk], w_sb[:, c, col0:col0 + 128],
                             src[:, c, :blk],
                             start=(c == 0), stop=(c == nchunk - 1))
        nc.scalar.activation(dst[:, sb * blk:(sb + 1) * blk], ps[:, :blk],
                             AF.Copy)
        if sum_ps is not None:
            sq = w_sq.tile([128, SB], BF16, tag="sqscratch", name="sqscratch")
            nc.scalar.activation(sq[:, :blk], ps[:, :blk], AF.Square)
            nc.tensor.matmul(sum_ps, ones_col[:, :],
                             dst[:, sb * blk:(sb + 1) * blk],
                             start=first, stop=last)
            nc.tensor.matmul(sq_ps, ones_bf[:, :], sq[:, :blk],
                             start=first, stop=last)

    for sb in range(NSB):
        if sb == 0:
            xtb = xtb0
        else:
            xtb = p_x.tile([128, NDC, SB], DT16, tag="xtb", name="xtb")
            strip_load(xtb, xT_r[:, :, sb * SB:(sb + 1) * SB], NDC)
        qsum = pp_stat.tile([1, SB], F32, tag="s0", name="s0")
        qsq = pp_stat.tile([1, SB], F32, tag="s1", name="s1")
        for i in range(HPC):
            proj_block(wq_sb, NDC, xtb, i * 128, qraw[i], sb, SB,
                       qsum[:, :], qsq[:, :], first=(i == 0),
                       last=(i == HPC - 1))
        stat_out(t["qin"].ap()[0:1, sb * SB:(sb + 1) * SB], qsum[:, :], SB)
        stat_out(t["qin"].ap()[1:2, sb * SB:(sb + 1) * SB], qsq[:, :], SB)

    if t["use_cc"]:
        nc.gpsimd.collective_compute(
            "AllReduce", Alu.add, replica_groups=t["groups"],
            ins=[t["qin"].ap().opt()], outs=[t["qout"].ap().opt()])
    else:
        nc.sync.dma_start(t["qout"].ap(), t["qin"].ap())

    # rope/LN constants can load while q projections run
    nc.sync.dma_start(swp[:, :], t["swapP"].ap())
    nc.sync.dma_start(cc[:, :], t["CC"].ap())
    nc.sync.dma_start(ssp[:, :], t["SSp"].ap())

    cm_qkv = tc.tile_pool(name="p_qkv", bufs=1, side="right")
    p_qkv = cm_qkv.__enter__()
    cm_rm = tc.tile_pool(name="rows_m", bufs=1, side="right")
    rows_m = cm_rm.__enter__()
    cm_wln = tc.tile_pool(name="w_ln", bufs=1, side="right")
    w_ln = cm_wln.__enter__()
    cm_wln2 = tc.tile_pool(name="w_ln2", bufs=2, side="right")
    w_ln2 = cm_wln2.__enter__()
    def moments(sum_src, sq_src, n, inv_scale, eps, length, r_rstd, r_nmr):
        """sum_src/sq_src: (dram_tensor, row). All math on [128, length/128]
        partition-parallel tiles; results DMA'd to lnr rows."""
        J = length // 128
        def rd(srct, row):
            tile_ = rows_m.tile([128, 16], F32, tag=f"m{row}", name=f"m{row}")
            ap = bass.AP(tensor=srct.ap().tensor, offset=row * length,
                         ap=[[J, 128], [1, J]])
            nc.sync.dma_start(tile_[:, :J], ap)
            return tile_
        a = rd(sum_src[0], sum_src[1])
        nc.vector.tensor_scalar_mul(a[:, :J], a[:, :J], inv_scale / n)
        b = rd(sq_src[0], sq_src[1])
        nc.vector.tensor_scalar_mul(b[:, :J], b[:, :J], inv_scale / n)
        c = rows_m.tile([128, 16], F32, tag="mc", name="mc")
        nc.vector.tensor_mul(c[:, :J], a[:, :J], a[:, :J])
        nc.vector.tensor_tensor(b[:, :J], b[:, :J], c[:, :J], Alu.subtract)
        nc.vector.tensor_scalar_add(b[:, :J], b[:, :J], eps)
        nc.scalar.activation(c[:, :J], b[:, :J], AF.Sqrt)
        nc.vector.reciprocal(c[:, :J], c[:, :J])
        d = rows_m.tile([128, 16], F32, tag="md", name="md")
        nc.vector.tensor_mul(d[:, :J], c[:, :J], c[:, :J])
        nc.vector.tensor_mul(d[:, :J], d[:, :J], b[:, :J])
        nc.vector.tensor_scalar(out=d[:, :J], in0=d[:, :J],
                                scalar1=-0.5, scalar2=1.5,
                                op0=Alu.mult, op1=Alu.add)
        nc.vector.tensor_mul(c[:, :J], c[:, :J], d[:, :J])
        nc.vector.tensor_mul(a[:, :J], a[:, :J], c[:, :J])
        nc.vector.tensor_scalar_mul(a[:, :J], a[:, :J], -1.0)
        ch = rows_m.tile([128, 16], DT16, tag="mch", name="mch")
        nc.vector.tensor_copy(ch[:, :J], c[:, :J])
        ah = rows_m.tile([128, 16], DT16, tag="mah", name="mah")
        nc.vector.tensor_copy(ah[:, :J], a[:, :J])
        out_r = bass.AP(tensor=t["lnr"].ap().tensor, offset=r_rstd * S,
                        ap=[[J, 128], [1, J]])
        nc.sync.dma_start(out_r, ch[:, :J])
        out_n = bass.AP(tensor=t["lnr"].ap().tensor, offset=r_nmr * S,
                        ap=[[J, 128], [1, J]])
        nc.sync.dma_start(out_n, ah[:, :J])

    def dma_bcast(dst, row, length):
        src_ap = bass.AP(tensor=t["lnr"].ap().tensor, offset=row * S,
                         ap=[[0, 128], [1, length]])
        nc.sync.dma_start(dst[:, :length], src_ap)

    def ln_type(raws, g_cols, b_cols, r_rstd, r_nmr, length, rope, fin_tag):
        rg = w_ln.tile([128, S], DT16, tag="bc_rg", name="bc_rg")
        dma_bcast(rg, r_rstd, length)
        ng = w_ln.tile([128, S], DT16, tag="bc_ng", name="bc_ng")
        dma_bcast(ng, r_nmr, length)
        fins = []
        for i, raw in enumerate(raws):
            t1 = w_ln2.tile([128, S], DT16, tag="lnt1", name="lnt1")
            nc.vector.tensor_mul(t1[:, :length], raw[:, :length],
                                 rg[:, :length])
            nc.vector.tensor_add(t1[:, :length], t1[:, :length],
                                 ng[:, :length])
            nc.vector.tensor_scalar(out=t1[:, :length], in0=t1[:, :length],
                                    scalar1=g_cols[:, i:i + 1],
                                    scalar2=b_cols[:, i:i + 1],
                                    op0=Alu.mult, op1=Alu.add)
            fin = p_qkv.tile([128, length], DT16, tag=f"{fin_tag}{i}",
                             name=f"{fin_tag}{i}")
            if not rope:
                nc.vector.tensor_copy(fin[:, :length], t1[:, :length])
                fins.append(fin)
                continue
            sw = w_ln2.tile([128, S], DT16, tag="swap", name="swap")
            for j in range(0, length, SB):
                ps = pp_proj.tile([128, SB], F32, tag="tp", name="tp")
                nc.tensor.matmul(ps[:, :], swp[:, :], t1[:, j:j + SB],
                                 start=True, stop=True)
                nc.scalar.activation(sw[:, j:j + SB], ps[:, :], AF.Copy)
            nc.vector.tensor_mul(t1[:, :length], t1[:, :length],
                                 cc[:, :length])
            nc.vector.tensor_mul(sw[:, :length], sw[:, :length],
                                 ssp[:, :length])
            nc.vector.tensor_add(fin[:, :length], t1[:, :length],
                                 sw[:, :length])
            fins.append(fin)
        return fins

    moments((t["qout"], 0), (t["qout"], 1), H * HD, 1.0, EPS_QK, S, 0, 1)
    QT = ln_type(qraw, qg_sb, qb_sb, 0, 1, S, True, "QT")

    # ============ phase 1b: k, then v/y projections; split AllReduces ======
    for sb in range(NSB):
        xtb = p_x.tile([128, NDC, SB], DT16, tag="xtb", name="xtb")
        strip_load(xtb, xT_r[:, :, sb * SB:(sb + 1) * SB], NDC)
        ksum = pp_stat.tile([1, SB], F32, tag="s0", name="s0")
        ksq = pp_stat.tile([1, SB], F32, tag="s1", name="s1")
        for i in range(KVPC):
            proj_block(wk_sb, NDC, xtb, i * 128, kraw[i], sb, SB,
                       ksum[:, :], ksq[:, :], first=(i == 0),
                       last=(i == KVPC - 1))
        for i in range(KVPC):
            proj_block(wv_sb, NDC, xtb, i * 128, vraw[i], sb, SB)
        stat_out(t["kin"].ap()[0:1, sb * SB:(sb + 1) * SB], ksum[:, :], SB)
        stat_out(t["kin"].ap()[1:2, sb * SB:(sb + 1) * SB], ksq[:, :], SB)

    if t["use_cc"]:
        nc.gpsimd.collective_compute(
            "AllReduce", Alu.add, replica_groups=t["groups"],
            ins=[t["kin"].ap().opt()], outs=[t["kout"].ap().opt()])
    else:
        nc.sync.dma_start(t["kout"].ap(), t["kin"].ap())

    ysum = pp_stat.tile([1, SB], F32, tag="s0", name="s0")
    ysq = pp_stat.tile([1, SB], F32, tag="s1", name="s1")
    for i in range(YHPC):
        proj_block(wky_sb, NYC, yt, i * 128, ykraw[i], 0, YL,
                   ysum[:, :YL], ysq[:, :YL], first=(i == 0),
                   last=(i == YHPC - 1))
    for i in range(YHPC):
        proj_block(wvy_sb, NYC, yt, i * 128, yvraw[i], 0, YL)
    stat_out(t["kyin"].ap()[0:1, 0:YL], ysum[:, :YL], YL)
    stat_out(t["kyin"].ap()[1:2, 0:YL], ysq[:, :YL], YL)

    if t["use_cc"]:
        nc.gpsimd.collective_compute(
            "AllReduce", Alu.add, replica_groups=t["groups"],
            ins=[t["kyin"].ap().opt()], outs=[t["kyout"].ap().opt()])
    else:
        nc.sync.dma_start(t["kyout"].ap(), t["kyin"].ap())

    cm_stg.__exit__(None, None, None)
    cm_wsq.__exit__(None, None, None)
    cm_x.__exit__(None, None, None)
    cm_ph1.__exit__(None, None, None)

    # ============ k/ky LN + V transposes ============
    # ---- V transposes (overlap q-LN) ----
    vnat = [p_qkv.tile([128, NKC, 128], DT16, tag=f"vnat{i}", name=f"vnat{i}")
            for i in range(KVPC)]
    for i in range(KVPC):
        for c in range(NKC):
            tp = pp_proj.tile([128, 128], DT16, tag="tp", name="tp")
            nc.tensor.transpose(tp[:, :], vraw[i][:, c * 128:(c + 1) * 128],
                                ident[:, :])
            nc.scalar.activation(vnat[i][:, c, :], tp[:, :], AF.Copy)
    yvnat = [p_qkv.tile([128, NYKC, 128], DT16, tag=f"yvnat{i}",
                        name=f"yvnat{i}")
             for i in range(YHPC)]
    for i in range(YHPC):
        for c in range(NYKC):
            tp = pp_proj.tile([128, 128], DT16, tag="tp", name="tp")
            nc.tensor.transpose(tp[:, :], yvraw[i][:, c * 128:(c + 1) * 128],
                                ident[:, :])
            nc.scalar.activation(yvnat[i][:, c, :], tp[:, :], AF.Copy)


    moments((t["kout"], 0), (t["kout"], 1), KV * HD, 1.0, EPS_QK, S, 2, 3)
    KT = ln_type(kraw, kg_sb, kb_sb, 2, 3, S, True, "KT")
    moments((t["kyout"], 0), (t["kyout"], 1), KV * HD, 0.5, EPS_KY, YL, 4, 5)
    YKT = ln_type(ykraw, kyg_sb, kyb_sb, 4, 5, YL, False, "YKT")

    cm_wln2.__exit__(None, None, None)
    cm_wln.__exit__(None, None, None)
    cm_rm.__exit__(None, None, None)
    cm_psB.__exit__(None, None, None)
    cm_psA.__exit__(None, None, None)
    cm_raw.__exit__(None, None, None)

    # ============ phase 4: attention ============
    cm_out = tc.tile_pool(name="p_out", bufs=1)
    p_out = cm_out.__enter__()
    outT = [p_out.tile([128, S], DT16, tag=f"outT{h}", name=f"outT{h}")
            for h in range(HPC)]
    cm_wo = tc.tile_pool(name="p_wo", bufs=1)
    p_wo = cm_wo.__enter__()
    wo_sb = p_wo.tile([128, HPC, D], DT16, tag="wo", name="wo")
    nc.sync.dma_start(wo_sb[:, :, :],
                      t["wo"].ap().rearrange("(c p) m -> p c m", p=128))
    cm_wat = tc.tile_pool(name="w_at", bufs=3)
    w_at = cm_wat.__enter__()
    cm_pt = tc.tile_pool(name="w_pt", bufs=4)
    w_pt = cm_pt.__enter__()
    cm_ra = tc.tile_pool(name="rows_a", bufs=3)
    rows_a = cm_ra.__enter__()

    cm_sc = tc.tile_pool(name="pp_sc", bufs=2, space="PSUM")
    cm_pv = tc.tile_pool(name="pp_pv", bufs=1, space="PSUM")
    cm_smr = tc.tile_pool(name="pp_smr", bufs=1, space="PSUM")
    pp_sc = cm_sc.__enter__()
    pp_pv = cm_pv.__enter__()
    pp_smr = cm_smr.__enter__()

    def attend_chunks(h, qb_i, KT_h, vnat_h, nkc, mask_sb):
        q0 = qb_i * QB
        pv = pp_pv.tile([128, QB], F32, tag="pv", name="pv")
        acc = w_at.tile([128, QB], DT16, tag="acc", name="acc")
        for c in range(nkc):
            sc = pp_sc.tile([128, QB], F32, tag="sc", name="sc")
            pt = w_pt.tile([128, QB], DT16, tag="ptile", name="ptile")
            for j in range(0, QB, SB):
                nc.tensor.matmul(sc[:, j:j + SB],
                                 KT_h[:, c * 128:(c + 1) * 128],
                                 QT[h][:, q0 + j:q0 + j + SB],
                                 start=True, stop=True)
            nc.scalar.activation(pt[:, :], sc[:, :], AF.Exp,
                                 bias=mask_sb[:, c:c + 1])
            for j in range(0, QB, SB):
                nc.tensor.matmul(pv[:, j:j + SB], vnat_h[:, c, :],
                                 pt[:, j:j + SB],
                                 start=(c == 0), stop=(c == nkc - 1))
            if c == 0:
                nc.vector.tensor_copy(acc[:, :], pt[:, :])
            else:
                nc.vector.tensor_add(acc[:, :], acc[:, :], pt[:, :])
        return pv, acc

    def attend_tail(pv, acc, gate_ap):
        sm = pp_smr.tile([128, QB], F32, tag="smr", name="smr")
        for j in range(0, QB, SB):
            nc.tensor.matmul(sm[0:1, j:j + SB], ones_col[:, :],
                             acc[:, j:j + SB], start=True, stop=True)
        srow = rows_a.tile([1, QB], F32, tag="srow", name="srow")
        nc.vector.reciprocal(srow[:, :], sm[0:1, :])
        if gate_ap is not None:
            nc.vector.tensor_scalar_mul(srow[:, :], srow[:, :], gate_ap)
        rbf = rows_a.tile([1, QB], DT16, tag="rbf", name="rbf")
        nc.vector.tensor_copy(rbf[:, :], srow[:, :])
        pvb = w_at.tile([128, QB], DT16, tag="pvb", name="pvb")
        nc.vector.tensor_copy(pvb[:, :], pv[:, :])
        o = w_at.tile([128, QB], DT16, tag="oattn", name="oattn")
        for j in range(0, QB, SB):
            rps = pp_smr.tile([128, QB], F32, tag="smr", name="smr")
            nc.tensor.matmul(rps[:, :SB], ones_row[:, :], rbf[0:1, j:j + SB],
                             start=True, stop=True)
            nc.vector.tensor_mul(o[:, j:j + SB], pvb[:, j:j + SB],
                                 rps[:, :SB])
        return o

    for h in range(HPC):
        for qb_i in range(NQB):
            pvS, accS = attend_chunks(h, qb_i, KT[h // 2], vnat[h // 2],
                                      NKC, xm_sb)
            pvY, accY = attend_chunks(h, qb_i, YKT[h], yvnat[h], NYKC, ym_sb)
            o_self = attend_tail(pvS, accS, None)
            o_y = attend_tail(pvY, accY, tg_sb[0:1, h:h + 1])
            nc.gpsimd.tensor_add(outT[h][:, qb_i * QB:(qb_i + 1) * QB],
                                 o_self[:, :], o_y[:, :])

    cm_smr.__exit__(None, None, None)
    cm_pv.__exit__(None, None, None)
    cm_sc.__exit__(None, None, None)
    cm_ra.__exit__(None, None, None)
    cm_pt.__exit__(None, None, None)
    cm_wat.__exit__(None, None, None)
    cm_qkv.__exit__(None, None, None)

    # ============ phase 5: output projection ============
    cm_wout = tc.tile_pool(name="w_out", bufs=2)
    w_out = cm_wout.__enter__()
    cm_po = tc.tile_pool(name="pp_out", bufs=2, space="PSUM")
    pp_out = cm_po.__enter__()
    for st in range(NST):
        ps = pp_out.tile([128, D], F32, tag="po", name="po")
        for dc in range(HPC):
            for j in range(0, D, SB):
                nc.tensor.matmul(ps[:, j:j + SB],
                                 outT[dc][:, st * 128:(st + 1) * 128],
                                 wo_sb[:, dc, j:j + SB],
                                 start=(dc == 0), stop=(dc == HPC - 1))
        ob = w_out.tile([128, D], F32, tag="obuf", name="obuf")
        nc.scalar.activation(ob[:, :], ps[:, :], AF.Copy)
        nc.sync.dma_start(t["out"].ap()[st * 128:(st + 1) * 128, :], ob[:, :])
    cm_po.__exit__(None, None, None)
    cm_wout.__exit__(None, None, None)
    cm_wo.__exit__(None, None, None)

    cm_out.__exit__(None, None, None)
    cm_consts.__exit__(None, None, None)


def _perm_cols(ncols):
    p = np.arange(ncols).reshape(-1, HD)
    return np.concatenate([p[:, 0::2], p[:, 1::2]], axis=1).reshape(-1)


def _prep_core_inputs(inputs, core):
    b, g = core // TP, core % TP
    f32 = np.float32
    x = np.asarray(inputs["x"], f32)
    y = np.asarray(inputs["y"], f32)

    qcols = np.arange(g * QW, (g + 1) * QW)
    kcols = np.arange(g * KW, (g + 1) * KW)
    y0 = (4 * g % 8) * HD
    ycols = np.arange(y0, y0 + YW)
    qperm = qcols[_perm_cols(QW)]
    kperm = kcols[_perm_cols(KW)]
    yperm = ycols[_perm_cols(YW)]

    scale = 1.0 / np.sqrt(HD)
    qg = (np.asarray(inputs["q_norm_g"], f32) * scale)[qperm]
    qb = (np.asarray(inputs["q_norm_b"], f32) * scale)[qperm]
    kg = np.asarray(inputs["k_norm_g"], f32)[kperm]
    kb = np.asarray(inputs["k_norm_b"], f32)[kperm]
    kyg = np.asarray(inputs["ky_norm_g"], f32)[yperm]
    kyb = np.asarray(inputs["ky_norm_b"], f32)[yperm]

    cos = np.asarray(inputs["freqs_cos"], f32)[b].T
    sin = np.asarray(inputs["freqs_sin"], f32)[b].T
    CCm = np.concatenate([cos, cos], 0)
    SSm = np.concatenate([-sin, sin], 0)
    swapP = np.zeros((128, 128), f32)
    swapP[np.arange(128), (np.arange(128) + 64) % 128] = 1.0

    xm = np.where(np.asarray(inputs["x_mask"][b]), 0.0, NEG).astype(f32)
    ym = np.where(np.asarray(inputs["y_mask"][b]), 0.0, NEG).astype(f32)
    tgv = np.tanh(np.asarray(inputs["gate"], f32)[4 * g:4 * g + 4])[None, :]

    bf = lambda a: np.ascontiguousarray(a).astype(NP16)
    return {
        "xT": bf(x[b].T), "yT": bf(y[b].T),
        "wq": bf(np.asarray(inputs["wq"], f32)[:, qperm]),
        "wk": bf(np.asarray(inputs["wk"], f32)[:, kperm]),
        "wv": bf(np.asarray(inputs["wv"], f32)[:, kcols]),
        "wky": bf(np.asarray(inputs["wk_y"], f32)[:, yperm]),
        "wvy": bf(np.asarray(inputs["wv_y"], f32)[:, ycols]),
        "wo": bf(np.asarray(inputs["wo"], f32)[qcols, :]),
        "CC": bf(CCm), "SSp": bf(SSm), "swapP": bf(swapP),
        "qgc": np.ascontiguousarray(qg.reshape(HPC, HD).T).astype(f32),
        "kgc": np.ascontiguousarray(kg.reshape(KVPC, HD).T).astype(f32),
        "kygc": np.ascontiguousarray(kyg.reshape(YHPC, HD).T).astype(f32),
        "qb": np.ascontiguousarray(qb.reshape(HPC, HD).T).astype(f32),
        "kb": np.ascontiguousarray(kb.reshape(KVPC, HD).T).astype(f32),
        "kyb": np.ascontiguousarray(kyb.reshape(YHPC, HD).T).astype(f32),
        "xmask": np.ascontiguousarray(xm.reshape(NKC, 128).T).astype(f32),
        "ymask": np.ascontiguousarray(ym.reshape(NYKC, 128).T).astype(f32),
        "tg": np.ascontiguousarray(tgv).astype(f32),
    }


def _get_runner():
    global _RUNNER
    if _RUNNER is None:
        _RUNNER = _build_program()
    return _RUNNER


def _get_exec():
    """Build (once) a cached jitted shard_map executable for the program."""
    global _EXEC
    if _EXEC is None:
        import jax
        from jax.experimental.shard_map import shard_map
        from jax.sharding import Mesh, NamedSharding, PartitionSpec

        nc = _get_runner()
        from concourse import bass2jax as b2j
        b2j.install_neuronx_cc_hook()

        pname = (nc.partition_id_tensor.name
                 if nc.partition_id_tensor else None)
        in_names, out_names, out_avals = [], [], []
        for alloc in nc.m.functions[0].allocations:
            if not isinstance(alloc, mybir.MemoryLocationSet):
                continue
            name = alloc.memorylocations[0].name
            if alloc.kind == "ExternalInput":
                if name != pname:
                    in_names.append(name)
            elif alloc.kind == "ExternalOutput":
                out_names.append(name)
                out_avals.append(jax.core.ShapedArray(
                    tuple(alloc.tensor_shape), mybir.dt.np(alloc.dtype)))
        n_params = len(in_names)
        all_in = list(in_names + out_names)
        if pname is not None:
            all_in.append(pname)
        all_in = tuple(all_in)
        donate = tuple(range(n_params, n_params + len(out_names)))

        def _body(*args):
            operands = list(args)
            if pname is not None:
                operands.append(b2j.partition_id_tensor())
            outs = b2j._bass_exec_p.bind(
                *operands, out_avals=tuple(out_avals), in_names=all_in,
                out_names=tuple(out_names),
                lowering_input_output_aliases=(),
                sim_require_finite=True, sim_require_nnan=True, nc=nc)
            return tuple(outs)

        devices = jax.devices()[:N_CORES]
        mesh = Mesh(np.asarray(devices), ("core",))
        nin = n_params + len(out_names)
        sharded = jax.jit(
            shard_map(_body, mesh=mesh,
                      in_specs=(PartitionSpec("core"),) * nin,
                      out_specs=(PartitionSpec("core"),) * len(out_names),
                      check_rep=False),
            donate_argnums=donate, keep_unused=True)
        shd = NamedSharding(mesh, PartitionSpec("core"))
        mk0 = [jax.jit(lambda a=a: __import__("jax.numpy", fromlist=["x"]
                                              ).zeros((N_CORES * a.shape[0],)
                                                      + a.shape[1:], a.dtype),
                       out_shardings=shd) for a in out_avals]
        _EXEC = (sharded, in_names, out_names, out_avals, shd, mk0)
    return _EXEC


def _concat_inputs(in_maps):
    sharded, in_names, out_names, out_avals, shd, mk0 = _get_exec()
    return [np.concatenate([np.asarray(in_maps[c][nm])
                            for c in range(N_CORES)], axis=0)
            for nm in in_names]


def _exec(concat_in, device_put=False):
    """Run once; returns {name: full concatenated np array}."""
    import jax
    sharded, in_names, out_names, out_avals, shd, mk0 = _get_exec()
    if device_put:
        concat_in = [jax.device_put(a, shd) for a in concat_in]
    outs = sharded(*concat_in, *[f() for f in mk0])
    return dict(zip(out_names, outs))


def run_on_cores(in_maps, trace=False):
    nc = _get_runner()
    return bass_utils.run_bass_kernel_spmd(
        nc, in_maps, core_ids=list(range(N_CORES)), trace=trace)


def kernel(**inputs):
    in_maps = [_prep_core_inputs(inputs, c) for c in range(N_CORES)]
    outs = _exec(_concat_inputs(in_maps))
    o = np.asarray(outs["out"]).reshape(N_CORES, S, D)
    out = np.zeros((B, S, D), np.float32)
    for c in range(N_CORES):
        out[c // TP] += o[c]
    return out



# revision 4
# speedup vs baseline: 1.0185x; 1.0185x over previous
"""Sharded attention kernel v2 for Trainium2 (8 NeuronCores, Bass/Tile).

Module: x->(wq,wk,wv) qk-norm + rope + GQA self-attn (+) gated cross-attn
over y->(wk_y,wv_y), then wo.  B=2, S=2048, D=2048, H=16, KV=8, HD=128,
YL=256, YD=1024.

Sharding (v2): core c owns the GQA pair {2c, 2c+1} of q heads for BOTH
batches (kv head c, y-kv heads {2c%8, 2c%8+1}).  Both batches on every
core makes the program symmetric, so batch 1's masked key tail (keys
1536..2047 when x_len=3S/4) is skipped on every core: 12 of 16 self-attn
key chunks.  wo is row-sharded; each core writes fp16 partials for both
batches, summed on the host.  The q/k/ky layernorm stats are (sum, sumsq)
partials AllReduced over all 8 cores, ONE COLLECTIVE PER BATCH so batch
0's LN and attention overlap batch 1's projections (ky double-counts by
2, folded into inv_scale).

Engine budget: PE does only the real matmuls (projections, scores, PV,
wo).  Softmax denominators come from gpsimd partition_all_reduce on the
otherwise idle Pool engine; the cross-attn gate is folded into wv_y on
the host; V is projected directly in [token, hd] layout (no transposes);
the rope half-swap is a partition-shifted SBUF->SBUF DMA.  wo is
interleaved into the attention stream per query block, and its PSUM ->
SBUF fp16 conversion copies are split between Act and DVE.
"""
import sys

sys.path.insert(0, "/opt/trn_rl_repo")

import numpy as np

import concourse.bass as bass  # noqa: F401
import concourse.tile as tile
from concourse import bacc, mybir, bass_isa
from concourse import bass_utils  # noqa: F401

DT16 = mybir.dt.float16
F32 = mybir.dt.float32
NP16 = np.float16

B, S, D, H, KV, YL, YD, HD = 2, 2048, 2048, 16, 8, 256, 1024, 128
N_CORES = 8
QH = 2                              # q heads per batch per core (GQA pair)
QW, KW, YW = QH * HD, HD, QH * HD   # 256, 128, 256 weight cols
NDC, NYC = D // 128, YD // 128      # contraction chunks: 16, 8
NSB, SB = 4, 512                    # seq blocks for projections
NKC0 = S // 128                     # 16 self key chunks (batch 0)
NYKC = YL // 128                    # 2 cross key chunks
QB = 512                            # query block (x2 heads = 1024 free)
NQB = S // QB                       # 4 query blocks
EPS_QK, EPS_KY = 1e-5, 1e-6
NEG = -1.0e30

_RUNNERS = {}
_EXECS = {}


def _build_program(nkc1=12, use_cc=True):
    nc = bacc.Bacc("TRN2", target_bir_lowering=False, debug=False,
                   num_devices=N_CORES if use_cc else 1)

    def din(name, shape, dt=DT16):
        return nc.dram_tensor(name, shape, dt, kind="ExternalInput")

    t = dict(
        xT=din("xT", [B, D, S]),
        yT=din("yT", [B, YD, YL]),
        wq=din("wq", [D, QW]),
        wk=din("wk", [D, KW]),
        wv=din("wv", [D, KW]),
        wky=din("wky", [YD, YW]),
        wvy=din("wvy", [YD, YW]),
        wo=din("wo", [QW, D]),
        CC=din("CC", [B, 128, S]),
        SSp=din("SSp", [B, 128, S]),
        qgc=din("qgc", [128, QH], F32),
        kgc=din("kgc", [128, 1], F32),
        kygc=din("kygc", [128, QH], F32),
        qb=din("qb", [128, QH], F32),
        kb=din("kb", [128, 1], F32),
        kyb=din("kyb", [128, QH], F32),
        xmask=din("xmask", [128, B * NKC0], F32),
        ymask=din("ymask", [128, B * NYKC], F32),
        out=nc.dram_tensor("out", [B, S, D], DT16, kind="ExternalOutput"),
        qin=nc.dram_tensor("qin", [4, S], F32),
        qout=nc.dram_tensor("qout", [4, S], F32),
        kin=nc.dram_tensor("kin", [4, S], F32),
        kout=nc.dram_tensor("kout", [4, S], F32),
        kyin=nc.dram_tensor("kyin", [4, YL], F32),
        kyout=nc.dram_tensor("kyout", [4, YL], F32),
        lnr=nc.dram_tensor("lnr", [12, S], DT16),
        groups=[list(range(N_CORES))],
        use_cc=use_cc,
        nkc=[NKC0, nkc1],
    )

    with tile.TileContext(nc) as tc:
        _emit(nc, tc, t)
    nc.compile()
    return nc


def _emit(nc, tc, t):
    AF = mybir.ActivationFunctionType
    Alu = mybir.AluOpType
    RED = bass_isa.ReduceOp

    cm_consts = tc.tile_pool(name="consts", bufs=1)
    consts = cm_consts.__enter__()

    # small-constant tiles; DMAs are issued after batch-0 projection
    # emission so they stay clear of the startup x/weight transfers
    qg_sb = consts.tile([128, QH], F32, tag="qgc", name="qgc")
    kg_sb = consts.tile([128, 1], F32, tag="kgc", name="kgc")
    kyg_sb = consts.tile([128, QH], F32, tag="kygc", name="kygc")
    qb_sb = consts.tile([128, QH], F32, tag="qb", name="qb")
    kb_sb = consts.tile([128, 1], F32, tag="kb", name="kb")
    kyb_sb = consts.tile([128, QH], F32, tag="kyb", name="kyb")
    xm_sb = consts.tile([128, B * NKC0], F32, tag="xm", name="xm")
    ym_sb = consts.tile([128, B * NYKC], F32, tag="ym", name="ym")
    cc_sb = [consts.tile([128, S], DT16, tag=f"cc{b}", name=f"cc{b}")
             for b in range(B)]
    ssp_sb = [consts.tile([128, S], DT16, tag=f"ssp{b}", name=f"ssp{b}")
              for b in range(B)]

    def load_consts():
        nc.gpsimd.dma_start(qg_sb[:, :], t["qgc"].ap())
        nc.gpsimd.dma_start(kg_sb[:, :], t["kgc"].ap())
        nc.gpsimd.dma_start(kyg_sb[:, :], t["kygc"].ap())
        nc.gpsimd.dma_start(qb_sb[:, :], t["qb"].ap())
        nc.gpsimd.dma_start(kb_sb[:, :], t["kb"].ap())
        nc.gpsimd.dma_start(kyb_sb[:, :], t["kyb"].ap())
        nc.gpsimd.dma_start(xm_sb[:, :], t["xmask"].ap())
        nc.gpsimd.dma_start(ym_sb[:, :], t["ymask"].ap())
        for b in range(B):
            nc.gpsimd.dma_start(cc_sb[b][:, :], t["CC"].ap()[b])
            nc.gpsimd.dma_start(ssp_sb[b][:, :], t["SSp"].ap()[b])

    load_consts()

    # ---------------- pools ----------------
    cm_raw = tc.tile_pool(name="p_raw", bufs=1)
    p_raw = cm_raw.__enter__()
    cm_w = tc.tile_pool(name="p_w", bufs=1)
    p_w = cm_w.__enter__()
    cm_x = tc.tile_pool(name="p_x", bufs=2)
    p_x = cm_x.__enter__()
    cm_sq = tc.tile_pool(name="w_sq", bufs=3)
    w_sq = cm_sq.__enter__()
    cm_stat = tc.tile_pool(name="w_stat", bufs=3)
    w_stat = cm_stat.__enter__()

    cm_psA = tc.tile_pool(name="pp_proj", bufs=2, space="PSUM")
    pp_proj = cm_psA.__enter__()
    cm_psV = tc.tile_pool(name="pp_v", bufs=2, space="PSUM")
    pp_v = cm_psV.__enter__()

    xT_r = [t["xT"].ap()[b].rearrange("(c p) s -> p c s", p=128)
            for b in range(B)]

    # first x block + wq strips lead the DMA queue for fast start
    wq_sb = p_w.tile([128, NDC, QW], DT16, tag="wq", name="wq")
    wq_r = t["wq"].ap().rearrange("(c p) m -> p c m", p=128)
    xtb0 = p_x.tile([128, NDC, SB], DT16, tag="xtb", name="xtb")
    for s0, s1 in ((0, 1), (1, 2), (2, 4), (4, 8), (8, 16)):
        nc.sync.dma_start(xtb0[:, s0:s1, :], xT_r[0][:, s0:s1, 0:SB])
        nc.sync.dma_start(wq_sb[:, s0:s1, :], wq_r[:, s0:s1, :])
    del wq_r
    wk_sb = p_w.tile([128, NDC, KW], DT16, tag="wk", name="wk")
    wk_r = t["wk"].ap().rearrange("(c p) m -> p c m", p=128)
    wv_sb = p_w.tile([128, NDC, KW], DT16, tag="wv", name="wv")
    wv_r = t["wv"].ap().rearrange("(c p) m -> p c m", p=128)
    for s0 in range(0, NDC, 8):
        nc.sync.dma_start(wk_sb[:, s0:s0 + 8, :], wk_r[:, s0:s0 + 8, :])
        nc.sync.dma_start(wv_sb[:, s0:s0 + 8, :], wv_r[:, s0:s0 + 8, :])
    yt = [p_w.tile([128, NYC, YL], DT16, tag=f"yt{b}", name=f"yt{b}")
          for b in range(B)]
    wky_sb = p_w.tile([128, NYC, YW], DT16, tag="wky", name="wky")
    wvy_sb = p_w.tile([128, NYC, YW], DT16, tag="wvy", name="wvy")
    nc.gpsimd.dma_start(wky_sb[:, :, :],
                        t["wky"].ap().rearrange("(c p) m -> p c m", p=128))
    nc.gpsimd.dma_start(wvy_sb[:, :, :],
                        t["wvy"].ap().rearrange("(c p) m -> p c m", p=128))
    for bb in range(B):
        nc.gpsimd.dma_start(yt[bb][:, :, :],
                            t["yT"].ap()[bb].rearrange("(c p) s -> p c s",
                                                       p=128))

    qraw = [[p_raw.tile([128, S], DT16, tag=f"qraw{b}{i}",
                        name=f"qraw{b}{i}") for i in range(QH)]
            for b in range(B)]
    kraw = [p_raw.tile([128, S], DT16, tag=f"kraw{b}", name=f"kraw{b}")
            for b in range(B)]
    ykraw = [p_raw.tile([128, QH, YL], DT16, tag=f"ykraw{b}",
                        name=f"ykraw{b}") for b in range(B)]

    cm_qkv = tc.tile_pool(name="p_qkv", bufs=1, side="right")
    p_qkv = cm_qkv.__enter__()
    QT = [[p_qkv.tile([128, S], DT16, tag=f"QT{b}{i}", name=f"QT{b}{i}")
           for i in range(QH)] for b in range(B)]
    KT = [p_qkv.tile([128, S], DT16, tag=f"KT{b}", name=f"KT{b}")
          for b in range(B)]
    vnat = [p_qkv.tile([128, NKC0, 128], DT16, tag=f"vnat{b}",
                       name=f"vnat{b}") for b in range(B)]
    YKT = [p_qkv.tile([128, QH, YL], DT16, tag=f"YKT{b}", name=f"YKT{b}")
           for b in range(B)]
    yvnat = [p_qkv.tile([128, NYKC, YW], DT16, tag=f"yvnat{b}",
                        name=f"yvnat{b}") for b in range(B)]

    cm_rm = tc.tile_pool(name="rows_m", bufs=1, side="right")
    rows_m = cm_rm.__enter__()
    cm_wln = tc.tile_pool(name="w_ln", bufs=1, side="right")
    w_ln = cm_wln.__enter__()
    cm_wln2 = tc.tile_pool(name="w_ln2", bufs=1, side="right")
    w_ln2 = cm_wln2.__enter__()

    def stat_to_row(dram, row, col0, blk, src_f16):
        """partition_all_reduce src [128, blk] f16 -> row0 -> dram row."""
        st = w_stat.tile([128, SB], F32, tag="st", name="st")
        nc.gpsimd.partition_all_reduce(st[:, :blk], src_f16, 128, RED.add)
        nc.gpsimd.dma_start(dram.ap()[row:row + 1, col0:col0 + blk],
                            st[0:1, :blk])

    def proj_batch(b, sbs=range(NSB)):
        for sb in sbs:
            if b == 0 and sb == 0:
                xtb = xtb0
            else:
                xtb = p_x.tile([128, NDC, SB], DT16, tag="xtb", name="xtb")
                for s0 in range(0, NDC, 8):
                    nc.sync.dma_start(
                        xtb[:, s0:s0 + 8, :],
                        xT_r[b][:, s0:s0 + 8, sb * SB:(sb + 1) * SB])
            sl = slice(sb * SB, (sb + 1) * SB)
            # q projections (2 head blocks)
            for i in range(QH):
                ps = pp_proj.tile([128, SB], F32, tag="proj", name="proj")
                for c in range(NDC):
                    nc.tensor.matmul(ps[:, :],
                                     wq_sb[:, c, i * 128:(i + 1) * 128],
                                     xtb[:, c, :], start=(c == 0),
                                     stop=(c == NDC - 1))
                nc.scalar.activation(qraw[b][i][:, sl], ps[:, :], AF.Copy)
            # k projection
            ps = pp_proj.tile([128, SB], F32, tag="proj", name="proj")
            for c in range(NDC):
                nc.tensor.matmul(ps[:, :], wk_sb[:, c, :], xtb[:, c, :],
                                 start=(c == 0), stop=(c == NDC - 1))
            nc.scalar.activation(kraw[b][:, sl], ps[:, :], AF.Copy)
            # v direct [token, hd] layout
            for s4 in range(4):
                ck = sb * 4 + s4
                psv = pp_v.tile([128, KW], F32, tag="pv", name="pv")
                for c in range(NDC):
                    nc.tensor.matmul(
                        psv[:, :], xtb[:, c, s4 * 128:(s4 + 1) * 128],
                        wv_sb[:, c, :], start=(c == 0), stop=(c == NDC - 1))
                nc.scalar.activation(vnat[b][:, ck, :], psv[:, :], AF.Copy)
            # stats: q sum/sumsq, k sum/sumsq (DVE squares, Pool reduce)
            s01 = w_sq.tile([128, SB], DT16, tag="sq", name="sq")
            nc.vector.tensor_tensor(s01[:, :], qraw[b][0][:, sl],
                                    qraw[b][1][:, sl], Alu.add)
            stat_to_row(t["qin"], 2 * b, sb * SB, SB, s01[:, :])
            sq0 = w_sq.tile([128, SB], DT16, tag="sq", name="sq")
            nc.vector.tensor_tensor(sq0[:, :], qraw[b][0][:, sl],
                                    qraw[b][0][:, sl], Alu.mult)
            sq1 = w_sq.tile([128, SB], DT16, tag="sq", name="sq")
            nc.vector.tensor_tensor(sq1[:, :], qraw[b][1][:, sl],
                                    qraw[b][1][:, sl], Alu.mult)
            nc.vector.tensor_tensor(sq0[:, :], sq0[:, :], sq1[:, :],
                                    Alu.add)
            stat_to_row(t["qin"], 2 * b + 1, sb * SB, SB, sq0[:, :])
            stat_to_row(t["kin"], 2 * b, sb * SB, SB, kraw[b][:, sl])
            sqk = w_sq.tile([128, SB], DT16, tag="sq", name="sq")
            nc.vector.tensor_tensor(sqk[:, :], kraw[b][:, sl],
                                    kraw[b][:, sl], Alu.mult)
            stat_to_row(t["kin"], 2 * b + 1, sb * SB, SB, sqk[:, :])

    def proj_y(b):
        for i in range(QH):
            ps = pp_proj.tile([128, SB], F32, tag="proj", name="proj")
            for c in range(NYC):
                nc.tensor.matmul(ps[:, :YL],
                                 wky_sb[:, c, i * 128:(i + 1) * 128],
                                 yt[b][:, c, :], start=(c == 0),
                                 stop=(c == NYC - 1))
            nc.scalar.activation(ykraw[b][:, i, :], ps[:, :YL], AF.Copy)
        for ck in range(NYKC):
            psv = pp_proj.tile([128, SB], F32, tag="proj", name="proj")
            for c in range(NYC):
                nc.tensor.matmul(
                    psv[:, :YW], yt[b][:, c, ck * 128:(ck + 1) * 128],
                    wvy_sb[:, c, :], start=(c == 0), stop=(c == NYC - 1))
            nc.scalar.activation(yvnat[b][:, ck, :], psv[:, :YW], AF.Copy)
        s01 = w_sq.tile([128, SB], DT16, tag="sq", name="sq")
        nc.vector.tensor_tensor(s01[:, :YL], ykraw[b][:, 0, :],
                                ykraw[b][:, 1, :], Alu.add)
        stat_to_row(t["kyin"], 2 * b, 0, YL, s01[:, :YL])
        sq0 = w_sq.tile([128, SB], DT16, tag="sq", name="sq")
        nc.vector.tensor_tensor(sq0[:, :YL], ykraw[b][:, 0, :],
                                ykraw[b][:, 0, :], Alu.mult)
        sq1 = w_sq.tile([128, SB], DT16, tag="sq", name="sq")
        nc.vector.tensor_tensor(sq1[:, :YL], ykraw[b][:, 1, :],
                                ykraw[b][:, 1, :], Alu.mult)
        nc.vector.tensor_tensor(sq0[:, :YL], sq0[:, :YL], sq1[:, :YL],
                                Alu.add)
        stat_to_row(t["kyin"], 2 * b + 1, 0, YL, sq0[:, :YL])

    def all_reduce_batch(b):
        for src, dst in (("qin", "qout"), ("kin", "kout"),
                         ("kyin", "kyout")):
            if t["use_cc"]:
                nc.gpsimd.collective_compute(
                    "AllReduce", Alu.add, replica_groups=t["groups"],
                    ins=[t[src].ap()[2 * b:2 * b + 2].opt()],
                    outs=[t[dst].ap()[2 * b:2 * b + 2].opt()])
            else:
                nc.gpsimd.dma_start(t[dst].ap()[2 * b:2 * b + 2],
                                    t[src].ap()[2 * b:2 * b + 2])

    def moments(src, b, n, inv_scale, eps, length, r_rstd, r_nmr):
        """src rows (2b: sum, 2b+1: sumsq) -> lnr rows r_rstd, r_nmr."""
        J = length // 128

        def rd(row):
            tile_ = rows_m.tile([128, 16], F32, tag=f"m{row % 2}",
                                name=f"m{row % 2}")
            ap = bass.AP(tensor=src.ap().tensor, offset=row * length,
                         ap=[[J, 128], [1, J]])
            nc.sync.dma_start(tile_[:, :J], ap)
            return tile_
        a = rd(2 * b)
        nc.vector.tensor_scalar_mul(a[:, :J], a[:, :J], inv_scale / n)
        bb = rd(2 * b + 1)
        nc.vector.tensor_scalar_mul(bb[:, :J], bb[:, :J], inv_scale / n)
        c = rows_m.tile([128, 16], F32, tag="mc", name="mc")
        nc.vector.tensor_mul(c[:, :J], a[:, :J], a[:, :J])
        nc.vector.tensor_tensor(bb[:, :J], bb[:, :J], c[:, :J],
                                Alu.subtract)
        nc.vector.tensor_scalar_add(bb[:, :J], bb[:, :J], eps)
        # rstd = rsqrt(var+eps), DVE-only (keeps Act on the exp/copy
        # table): seed 0.44 + 0.38/v, then 4 Newton steps
        nc.vector.reciprocal(c[:, :J], bb[:, :J])
        nc.vector.tensor_scalar(out=c[:, :J], in0=c[:, :J],
                                scalar1=0.38, scalar2=0.44,
                                op0=Alu.mult, op1=Alu.add)
        d = rows_m.tile([128, 16], F32, tag="md", name="md")
        for _ in range(4):
            nc.vector.tensor_mul(d[:, :J], c[:, :J], c[:, :J])
            nc.vector.tensor_mul(d[:, :J], d[:, :J], bb[:, :J])
            nc.vector.tensor_scalar(out=d[:, :J], in0=d[:, :J],
                                    scalar1=-0.5, scalar2=1.5,
                                    op0=Alu.mult, op1=Alu.add)
            nc.vector.tensor_mul(c[:, :J], c[:, :J], d[:, :J])
        nc.vector.tensor_mul(a[:, :J], a[:, :J], c[:, :J])
        nc.vector.tensor_scalar_mul(a[:, :J], a[:, :J], -1.0)
        ch = rows_m.tile([128, 16], DT16, tag="mch", name="mch")
        nc.vector.tensor_copy(ch[:, :J], c[:, :J])
        ah = rows_m.tile([128, 16], DT16, tag="mah", name="mah")
        nc.vector.tensor_copy(ah[:, :J], a[:, :J])
        out_r = bass.AP(tensor=t["lnr"].ap().tensor, offset=r_rstd * S,
                        ap=[[J, 128], [1, J]])
        nc.sync.dma_start(out_r, ch[:, :J])
        out_n = bass.AP(tensor=t["lnr"].ap().tensor, offset=r_nmr * S,
                        ap=[[J, 128], [1, J]])
        nc.sync.dma_start(out_n, ah[:, :J])

    def dma_bcast(dst, row, length):
        src_ap = bass.AP(tensor=t["lnr"].ap().tensor, offset=row * S,
                         ap=[[0, 128], [1, length]])
        nc.sync.dma_start(dst[:, :length], src_ap)

    def ln_rope(raw_ap, fin_ap, rg, ng, g_col, b_col, length, rope_b, eng):
        t1 = w_ln2.tile([128, S], DT16, tag="lnt1", name="lnt1")
        eng.tensor_mul(t1[:, :length], raw_ap, rg[:, :length])
        eng.tensor_add(t1[:, :length], t1[:, :length], ng[:, :length])
        nc.vector.tensor_scalar(out=t1[:, :length], in0=t1[:, :length],
                                scalar1=g_col, scalar2=b_col,
                                op0=Alu.mult, op1=Alu.add)
        if rope_b is None:
            nc.vector.tensor_copy(fin_ap, t1[:, :length])
            return
        sw = w_ln2.tile([128, S], DT16, tag="swap", name="swap")
        nc.sync.dma_start(sw[0:64, :length], t1[64:128, :length])
        nc.sync.dma_start(sw[64:128, :length], t1[0:64, :length])
        m1 = w_ln2.tile([128, S], DT16, tag="m1", name="m1")
        nc.vector.tensor_mul(m1[:, :length], t1[:, :length],
                             cc_sb[rope_b][:, :length])
        nc.vector.tensor_mul(sw[:, :length], sw[:, :length],
                             ssp_sb[rope_b][:, :length])
        nc.vector.tensor_add(fin_ap, m1[:, :length], sw[:, :length])

    def moments_batch(b):
        moments(t["qout"], b, H * HD, 1.0, EPS_QK, S, 2 * b, 2 * b + 1)
        moments(t["kyout"], b, KV * HD, 0.5, EPS_KY, YL, 8 + 2 * b,
                9 + 2 * b)
        moments(t["kout"], b, KV * HD, 1.0, EPS_QK, S, 4 + 2 * b,
                5 + 2 * b)

    def lnapply_q(b, eng):
        rg = w_ln.tile([128, S], DT16, tag="bc_rg", name="bc_rg")
        dma_bcast(rg, 2 * b, S)
        ng = w_ln.tile([128, S], DT16, tag="bc_ng", name="bc_ng")
        dma_bcast(ng, 2 * b + 1, S)
        for i in range(QH):
            ln_rope(qraw[b][i][:, :], QT[b][i][:, :], rg, ng,
                    qg_sb[:, i:i + 1], qb_sb[:, i:i + 1], S, b, eng)

    def lnapply_ky(b, eng):
        rg = w_ln.tile([128, S], DT16, tag="bc_rg", name="bc_rg")
        dma_bcast(rg, 8 + 2 * b, YL)
        ng = w_ln.tile([128, S], DT16, tag="bc_ng", name="bc_ng")
        dma_bcast(ng, 9 + 2 * b, YL)
        for i in range(QH):
            ln_rope(ykraw[b][:, i, :], YKT[b][:, i, :], rg, ng,
                    kyg_sb[:, i:i + 1], kyb_sb[:, i:i + 1], YL, None, eng)

    def lnapply_k(b, eng):
        rg = w_ln.tile([128, S], DT16, tag="bc_rg", name="bc_rg")
        dma_bcast(rg, 4 + 2 * b, S)
        ng = w_ln.tile([128, S], DT16, tag="bc_ng", name="bc_ng")
        dma_bcast(ng, 5 + 2 * b, S)
        ln_rope(kraw[b][:, :], KT[b][:, :], rg, ng,
                kg_sb[:, 0:1], kb_sb[:, 0:1], S, b, eng)

    # outY for batch 0 lives in the long-lived right pool: written by the
    # cross-attn groups interleaved into batch-1 projections, read at the
    # batch-0 self-attn tails.
    outY = [[p_qkv.tile([128, S], DT16, tag=f"outY0{h}", name=f"outY0{h}")
             for h in range(QH)], [None, None]]
    outT = [[None, None], [None, None]]
    P = {}
    ncopy = [0]

    def attend(b, qb_i, keys_T, vals, nkc, mask_sb, mask_col0, cross,
               lag=9):
        """Head-paired attention for query block qb_i of batch b.

        PV matmuls lag the score/exp stream by `lag` chunks so the PE
        in-order queue has score work while the previous group's pv PSUM
        bank drains through its denominator chain.
        """
        q0 = qb_i * QB
        lag = min(lag, nkc - 1)
        pv = P["pv"].tile([128, 2 * QB], F32, tag="pv", name="pv")
        acc = P["acc"].tile([128, 2 * QB], DT16, tag="acc", name="acc")
        pts = {}

        def pv_step(c):
            for h in range(QH):
                nc.tensor.matmul(pv[:, h * QB:(h + 1) * QB], vals(h, c),
                                 pts[c][:, h * QB:(h + 1) * QB],
                                 start=(c == 0), stop=(c == nkc - 1))
            del pts[c]

        for c in range(nkc):
            sc = P["sc"].tile([128, 2 * QB], F32, tag="sc", name="sc")
            pt = P["pt"].tile([128, 2 * QB], DT16, tag="ptile",
                              name="ptile")
            pts[c] = pt
            for h in range(QH):
                nc.tensor.matmul(sc[:, h * QB:(h + 1) * QB], keys_T(h, c),
                                 QT[b][h][:, q0:q0 + QB],
                                 start=True, stop=True)
            nc.scalar.activation(
                pt[:, :], sc[:, :], AF.Exp,
                bias=mask_sb[:, mask_col0 + c:mask_col0 + c + 1])
            if c >= lag:
                pv_step(c - lag)
            if c == 0:
                nc.vector.tensor_copy(acc[:, :], pt[:, :])
            else:
                nc.vector.tensor_add(acc[:, :], acc[:, :], pt[:, :])
        for c in range(nkc - lag, nkc):
            pv_step(c)
        den = P["den"].tile([128, 2 * QB], DT16, tag="den", name="den")
        nc.gpsimd.partition_all_reduce(den[:, :], acc[:, :], 128, RED.add)
        rden = P["den"].tile([128, 2 * QB], F32, tag="rden", name="rden")
        nc.vector.reciprocal(rden[:, :], den[:, :])
        for h in range(QH):
            dst = (outY if cross else outT)[b][h][:, q0:q0 + QB]
            nc.vector.tensor_mul(dst, pv[:, h * QB:(h + 1) * QB],
                                 rden[:, h * QB:(h + 1) * QB])
            if not cross:
                nc.vector.tensor_add(dst, dst, outY[b][h][:, q0:q0 + QB])

    def wo_block(b, qb_i):
        last = b == 1 and qb_i == 3
        for st in range(qb_i * 4, qb_i * 4 + 4):
            ob = P["ob"].tile([128, D], DT16, tag="obuf", name="obuf")
            for jc in range(4):
                pso = P["wo"].tile([128, 512], F32, tag="wops",
                                   name="wops")
                for h in range(QH):
                    nc.tensor.matmul(
                        pso[:, :], outT[b][h][:, st * 128:(st + 1) * 128],
                        P["wo_sb"][:, h, jc * 512:(jc + 1) * 512],
                        start=(h == 0), stop=(h == QH - 1))
                on_act = (jc % 2 == 0) if last else \
                    (ncopy[0] % 8 < (6 if b == 0 else 2))
                if on_act:
                    nc.scalar.activation(ob[:, jc * 512:(jc + 1) * 512],
                                         pso[:, :], AF.Copy)
                else:
                    nc.vector.tensor_copy(ob[:, jc * 512:(jc + 1) * 512],
                                          pso[:, :])
                ncopy[0] += 1
                if last and jc == 1:
                    nc.sync.dma_start(
                        t["out"].ap()[b][st * 128:(st + 1) * 128, 0:1024],
                        ob[:, 0:1024])
            if last:
                nc.sync.dma_start(
                    t["out"].ap()[b][st * 128:(st + 1) * 128, 1024:D],
                    ob[:, 1024:D])
            else:
                nc.sync.dma_start(
                    t["out"].ap()[b][st * 128:(st + 1) * 128, :], ob[:, :])

    def cross_g(b, qb_i, lag=1):
        attend(b, qb_i,
               lambda h, c, b=b: YKT[b][:, h, c * 128:(c + 1) * 128],
               lambda h, c, b=b: yvnat[b][:, c, h * 128:(h + 1) * 128],
               NYKC, ym_sb, b * NYKC, True, lag=lag)

    def self_g(b, qb_i):
        attend(b, qb_i,
               lambda h, c, b=b: KT[b][:, c * 128:(c + 1) * 128],
               lambda h, c, b=b: vnat[b][:, c, :],
               t["nkc"][b], xm_sb, b * NKC0, False)

    # ============ batch-0 projections ============
    proj_batch(0)
    proj_y(0)
    all_reduce_batch(0)
    moments_batch(0)     # DVE+Act(sqrt): overlaps remaining projections
    lnapply_q(0, nc.vector)
    lnapply_ky(0, nc.vector)
    lnapply_k(0, nc.vector)

    # ===== batch-1 projections with batch-0 cross-attn interleaved =====
    proj_batch(1, [0, 1])
    cm_cpt = tc.tile_pool(name="crs_pt", bufs=2)
    cm_cacc = tc.tile_pool(name="crs_acc", bufs=1)
    cm_cden = tc.tile_pool(name="crs_den", bufs=1)
    cm_csc = tc.tile_pool(name="crs_sc", bufs=1, space="PSUM")
    cm_cpv = tc.tile_pool(name="crs_pv", bufs=1, space="PSUM")
    P.update(pt=cm_cpt.__enter__(), acc=cm_cacc.__enter__(),
             den=cm_cden.__enter__(), sc=cm_csc.__enter__(),
             pv=cm_cpv.__enter__())
    cross_g(0, 0)
    cross_g(0, 1)
    proj_batch(1, [2])
    cross_g(0, 2)
    proj_batch(1, [3])
    cross_g(0, 3)
    proj_y(1)
    all_reduce_batch(1)
    moments_batch(1)     # all Act sqrt done before self-attn exps

    cm_cpv.__exit__(None, None, None)
    cm_csc.__exit__(None, None, None)
    cm_cden.__exit__(None, None, None)
    cm_cacc.__exit__(None, None, None)
    cm_cpt.__exit__(None, None, None)
    cm_psV.__exit__(None, None, None)
    cm_psA.__exit__(None, None, None)
    cm_stat.__exit__(None, None, None)
    cm_sq.__exit__(None, None, None)
    cm_x.__exit__(None, None, None)
    cm_w.__exit__(None, None, None)

    # ============ attention + wo ============
    cm_out = tc.tile_pool(name="p_out", bufs=1)
    p_out = cm_out.__enter__()
    for b in range(B):
        for h in range(QH):
            outT[b][h] = p_out.tile([128, S], DT16, tag=f"outT{b}{h}",
                                    name=f"outT{b}{h}")
    for h in range(QH):
        outY[1][h] = p_out.tile([128, S], DT16, tag=f"outY1{h}",
                                name=f"outY1{h}")
    cm_wo = tc.tile_pool(name="p_wo", bufs=1)
    p_wo = cm_wo.__enter__()
    wo_sb = p_wo.tile([128, QH, D], DT16, tag="wo", name="wo")
    nc.gpsimd.dma_start(wo_sb[:, :, :],
                        t["wo"].ap().rearrange("(c p) m -> p c m", p=128))
    cm_pt = tc.tile_pool(name="w_pt", bufs=10)
    cm_acc = tc.tile_pool(name="w_acc", bufs=2)
    cm_den = tc.tile_pool(name="w_den", bufs=2)
    cm_ob = tc.tile_pool(name="w_ob", bufs=2)
    cm_sc = tc.tile_pool(name="pp_sc", bufs=2, space="PSUM")
    cm_pv = tc.tile_pool(name="pp_pv", bufs=1, space="PSUM")
    cm_po = tc.tile_pool(name="pp_wo", bufs=2, space="PSUM")
    P.update(pt=cm_pt.__enter__(), acc=cm_acc.__enter__(),
             den=cm_den.__enter__(), ob=cm_ob.__enter__(),
             sc=cm_sc.__enter__(), pv=cm_pv.__enter__(),
             wo=cm_po.__enter__(), wo_sb=wo_sb)

    # self-attn with wo one query-block behind as in-order PE filler;
    # batch-1 LN applies (Pool+DVE) slot between batch-0 groups.
    self_g(0, 0)
    lnapply_q(1, nc.vector)
    self_g(0, 1)
    wo_block(0, 0)
    lnapply_ky(1, nc.vector)
    self_g(0, 2)
    wo_block(0, 1)
    lnapply_k(1, nc.vector)
    self_g(0, 3)
    wo_block(0, 2)
    cross_g(1, 0, lag=1)
    self_g(1, 0)
    wo_block(0, 3)
    cross_g(1, 1, lag=1)
    self_g(1, 1)
    wo_block(1, 0)
    cross_g(1, 2, lag=1)
    self_g(1, 2)
    wo_block(1, 1)
    cross_g(1, 3, lag=1)
    self_g(1, 3)
    wo_block(1, 2)
    wo_block(1, 3)

    cm_po.__exit__(None, None, None)
    cm_pv.__exit__(None, None, None)
    cm_sc.__exit__(None, None, None)
    cm_ob.__exit__(None, None, None)
    cm_den.__exit__(None, None, None)
    cm_acc.__exit__(None, None, None)
    cm_pt.__exit__(None, None, None)
    cm_wo.__exit__(None, None, None)
    cm_out.__exit__(None, None, None)
    cm_wln2.__exit__(None, None, None)
    cm_wln.__exit__(None, None, None)
    cm_rm.__exit__(None, None, None)
    cm_qkv.__exit__(None, None, None)
    cm_raw.__exit__(None, None, None)
    cm_consts.__exit__(None, None, None)


def _perm_cols(ncols):
    p = np.arange(ncols).reshape(-1, HD)
    return np.concatenate([p[:, 0::2], p[:, 1::2]], axis=1).reshape(-1)


def _prep_core_inputs(inputs, core):
    c = core
    f32 = np.float32
    x = np.asarray(inputs["x"], f32)
    y = np.asarray(inputs["y"], f32)

    qcols = np.arange(2 * c * HD, (2 * c + 2) * HD)
    kcols = np.arange(c * HD, (c + 1) * HD)
    y0 = ((2 * c) % KV) * HD
    ycols = np.arange(y0, y0 + 2 * HD)
    qperm = qcols[_perm_cols(2 * HD)]
    kperm = kcols[_perm_cols(HD)]
    yperm = ycols[_perm_cols(2 * HD)]

    scale = 1.0 / np.sqrt(HD)
    qg = (np.asarray(inputs["q_norm_g"], f32) * scale)[qperm]
    qb = (np.asarray(inputs["q_norm_b"], f32) * scale)[qperm]
    kg = np.asarray(inputs["k_norm_g"], f32)[kperm]
    kb = np.asarray(inputs["k_norm_b"], f32)[kperm]
    kyg = np.asarray(inputs["ky_norm_g"], f32)[yperm]
    kyb = np.asarray(inputs["ky_norm_b"], f32)[yperm]

    CCm = np.zeros((B, 128, S), f32)
    SSm = np.zeros((B, 128, S), f32)
    for b in range(B):
        cos = np.asarray(inputs["freqs_cos"], f32)[b].T
        sin = np.asarray(inputs["freqs_sin"], f32)[b].T
        CCm[b] = np.concatenate([cos, cos], 0)
        SSm[b] = np.concatenate([-sin, sin], 0)

    xm = np.where(np.asarray(inputs["x_mask"]), 0.0, NEG).astype(f32)
    ym = np.where(np.asarray(inputs["y_mask"]), 0.0, NEG).astype(f32)
    xmt = np.concatenate([xm[b].reshape(NKC0, 128).T for b in range(B)], 1)
    ymt = np.concatenate([ym[b].reshape(NYKC, 128).T for b in range(B)], 1)

    tg = np.tanh(np.asarray(inputs["gate"], f32))
    wvy = np.asarray(inputs["wv_y"], f32)[:, ycols].copy()
    wvy[:, 0:HD] *= tg[2 * c]
    wvy[:, HD:2 * HD] *= tg[2 * c + 1]

    bf = lambda a: np.ascontiguousarray(a).astype(NP16)
    return {
        "xT": bf(np.swapaxes(x, 1, 2)),
        "yT": bf(np.swapaxes(y, 1, 2)),
        "wq": bf(np.asarray(inputs["wq"], f32)[:, qperm]),
        "wk": bf(np.asarray(inputs["wk"], f32)[:, kperm]),
        "wv": bf(np.asarray(inputs["wv"], f32)[:, kcols]),
        "wky": bf(np.asarray(inputs["wk_y"], f32)[:, yperm]),
        "wvy": bf(wvy),
        "wo": bf(np.asarray(inputs["wo"], f32)[qcols, :]),
        "CC": bf(CCm), "SSp": bf(SSm),
        "qgc": np.ascontiguousarray(qg.reshape(QH, HD).T).astype(f32),
        "kgc": np.ascontiguousarray(kg.reshape(1, HD).T).astype(f32),
        "kygc": np.ascontiguousarray(kyg.reshape(QH, HD).T).astype(f32),
        "qb": np.ascontiguousarray(qb.reshape(QH, HD).T).astype(f32),
        "kb": np.ascontiguousarray(kb.reshape(1, HD).T).astype(f32),
        "kyb": np.ascontiguousarray(kyb.reshape(QH, HD).T).astype(f32),
        "xmask": np.ascontiguousarray(xmt).astype(f32),
        "ymask": np.ascontiguousarray(ymt).astype(f32),
    }


def _pick_variant(inputs):
    xm = np.asarray(inputs["x_mask"])
    if not xm[1, 12 * 128:].any():
        return 12
    return NKC0


def _get_runner(nkc1):
    if nkc1 not in _RUNNERS:
        _RUNNERS[nkc1] = _build_program(nkc1)
    return _RUNNERS[nkc1]


def _get_exec(nkc1):
    """Build (once) a cached jitted shard_map executable for the program."""
    if nkc1 not in _EXECS:
        import jax
        from jax.experimental.shard_map import shard_map
        from jax.sharding import Mesh, NamedSharding, PartitionSpec

        nc = _get_runner(nkc1)
        from concourse import bass2jax as b2j
        b2j.install_neuronx_cc_hook()

        pname = (nc.partition_id_tensor.name
                 if nc.partition_id_tensor else None)
        in_names, out_names, out_avals = [], [], []
        for alloc in nc.m.functions[0].allocations:
            if not isinstance(alloc, mybir.MemoryLocationSet):
                continue
            name = alloc.memorylocations[0].name
            if alloc.kind == "ExternalInput":
                if name != pname:
                    in_names.append(name)
            elif alloc.kind == "ExternalOutput":
                out_names.append(name)
                out_avals.append(jax.core.ShapedArray(
                    tuple(alloc.tensor_shape), mybir.dt.np(alloc.dtype)))
        n_params = len(in_names)
        all_in = list(in_names + out_names)
        if pname is not None:
            all_in.append(pname)
        all_in = tuple(all_in)
        donate = tuple(range(n_params, n_params + len(out_names)))

        def _body(*args):
            operands = list(args)
            if pname is not None:
                operands.append(b2j.partition_id_tensor())
            outs = b2j._bass_exec_p.bind(
                *operands, out_avals=tuple(out_avals), in_names=all_in,
                out_names=tuple(out_names),
                lowering_input_output_aliases=(),
                sim_require_finite=True, sim_require_nnan=True, nc=nc)
            return tuple(outs)

        devices = jax.devices()[:N_CORES]
        mesh = Mesh(np.asarray(devices), ("core",))
        nin = n_params + len(out_names)
        sharded = jax.jit(
            shard_map(_body, mesh=mesh,
                      in_specs=(PartitionSpec("core"),) * nin,
                      out_specs=(PartitionSpec("core"),) * len(out_names),
                      check_rep=False),
            donate_argnums=donate, keep_unused=True)
        shd = NamedSharding(mesh, PartitionSpec("core"))
        mk0 = [jax.jit(lambda a=a: __import__("jax.numpy", fromlist=["x"]
                                              ).zeros((N_CORES * a.shape[0],)
                                                      + a.shape[1:], a.dtype),
                       out_shardings=shd) for a in out_avals]
        _EXECS[nkc1] = (sharded, in_names, out_names, out_avals, shd, mk0)
    return _EXECS[nkc1]


def _concat_inputs(in_maps, nkc1):
    sharded, in_names, out_names, out_avals, shd, mk0 = _get_exec(nkc1)
    return [np.concatenate([np.asarray(in_maps[c][nm])
                            for c in range(N_CORES)], axis=0)
            for nm in in_names]


def _exec(concat_in, nkc1, device_put=False):
    import jax
    sharded, in_names, out_names, out_avals, shd, mk0 = _get_exec(nkc1)
    if device_put:
        concat_in = [jax.device_put(a, shd) for a in concat_in]
    outs = sharded(*concat_in, *[f() for f in mk0])
    return dict(zip(out_names, outs))


def kernel(**inputs):
    nkc1 = _pick_variant(inputs)
    in_maps = [_prep_core_inputs(inputs, c) for c in range(N_CORES)]
    outs = _exec(_concat_inputs(in_maps, nkc1), nkc1)
    o = np.asarray(outs["out"]).reshape(N_CORES, B, S, D)
    out = np.zeros((B, S, D), np.float32)
    for c in range(N_CORES):
        out += o[c].astype(np.float32)
    return out


# revision 5
# speedup vs baseline: 1.0244x; 1.0058x over previous
"""Sharded attention kernel v2 for Trainium2 (8 NeuronCores, Bass/Tile).

Module: x->(wq,wk,wv) qk-norm + rope + GQA self-attn (+) gated cross-attn
over y->(wk_y,wv_y), then wo.  B=2, S=2048, D=2048, H=16, KV=8, HD=128,
YL=256, YD=1024.

Sharding (v2): core c owns the GQA pair {2c, 2c+1} of q heads for BOTH
batches (kv head c, y-kv heads {2c%8, 2c%8+1}).  Both batches on every
core makes the program symmetric, so batch 1's masked key tail (keys
1536..2047 when x_len=3S/4) is skipped on every core: 12 of 16 self-attn
key chunks.  wo is row-sharded; each core writes fp16 partials for both
batches, summed on the host.  The q/k/ky layernorm stats are (sum, sumsq)
partials AllReduced over all 8 cores, ONE COLLECTIVE PER BATCH so batch
0's LN and attention overlap batch 1's projections (ky double-counts by
2, folded into inv_scale).

Engine budget: PE does only the real matmuls (projections, scores, PV,
wo).  Softmax denominators come from gpsimd partition_all_reduce on the
otherwise idle Pool engine; the cross-attn gate is folded into wv_y on
the host; V is projected directly in [token, hd] layout (no transposes);
the rope half-swap is a partition-shifted SBUF->SBUF DMA.  wo is
interleaved into the attention stream per query block, and its PSUM ->
SBUF fp16 conversion copies are split between Act and DVE.
"""
import sys

sys.path.insert(0, "/opt/trn_rl_repo")

import numpy as np

import concourse.bass as bass  # noqa: F401
import concourse.tile as tile
from concourse import bacc, mybir, bass_isa
from concourse import bass_utils  # noqa: F401

DT16 = mybir.dt.float16
F32 = mybir.dt.float32
NP16 = np.float16

B, S, D, H, KV, YL, YD, HD = 2, 2048, 2048, 16, 8, 256, 1024, 128
N_CORES = 8
QH = 2                              # q heads per batch per core (GQA pair)
QW, KW, YW = QH * HD, HD, QH * HD   # 256, 128, 256 weight cols
NDC, NYC = D // 128, YD // 128      # contraction chunks: 16, 8
NSB, SB = 4, 512                    # seq blocks for projections
NKC0 = S // 128                     # 16 self key chunks (batch 0)
NYKC = YL // 128                    # 2 cross key chunks
QB = 512                            # query block (x2 heads = 1024 free)
NQB = S // QB                       # 4 query blocks
EPS_QK, EPS_KY = 1e-5, 1e-6
NEG = -1.0e30

_RUNNERS = {}
_EXECS = {}


def _build_program(nkc1=12, use_cc=True):
    nc = bacc.Bacc("TRN2", target_bir_lowering=False, debug=False,
                   num_devices=N_CORES if use_cc else 1)

    def din(name, shape, dt=DT16):
        return nc.dram_tensor(name, shape, dt, kind="ExternalInput")

    t = dict(
        xT=din("xT", [B, D, S]),
        yT=din("yT", [B, YD, YL]),
        wq=din("wq", [D, QW]),
        wk=din("wk", [D, KW]),
        wv=din("wv", [D, KW]),
        wky=din("wky", [YD, YW]),
        wvy=din("wvy", [YD, YW]),
        wo=din("wo", [QW, D]),
        CC=din("CC", [B, 128, S]),
        SSp=din("SSp", [B, 128, S]),
        qgc=din("qgc", [128, QH], F32),
        kgc=din("kgc", [128, 1], F32),
        kygc=din("kygc", [128, QH], F32),
        qb=din("qb", [128, QH], F32),
        kb=din("kb", [128, 1], F32),
        kyb=din("kyb", [128, QH], F32),
        xmask=din("xmask", [128, B * NKC0], F32),
        ymask=din("ymask", [128, B * NYKC], F32),
        out=nc.dram_tensor("out", [B, S, D], DT16, kind="ExternalOutput"),
        qin=nc.dram_tensor("qin", [4, S], F32),
        qout=nc.dram_tensor("qout", [4, S], F32),
        kin=nc.dram_tensor("kin", [4, S], F32),
        kout=nc.dram_tensor("kout", [4, S], F32),
        kyin=nc.dram_tensor("kyin", [4, YL], F32),
        kyout=nc.dram_tensor("kyout", [4, YL], F32),
        lnr=nc.dram_tensor("lnr", [12, S], DT16),
        groups=[list(range(N_CORES))],
        use_cc=use_cc,
        nkc=[NKC0, nkc1],
    )

    with tile.TileContext(nc) as tc:
        _emit(nc, tc, t)
    nc.compile()
    return nc


def _emit(nc, tc, t):
    AF = mybir.ActivationFunctionType
    Alu = mybir.AluOpType
    RED = bass_isa.ReduceOp

    cm_consts = tc.tile_pool(name="consts", bufs=1)
    consts = cm_consts.__enter__()

    # small-constant tiles; DMAs are issued after batch-0 projection
    # emission so they stay clear of the startup x/weight transfers
    qg_sb = consts.tile([128, QH], F32, tag="qgc", name="qgc")
    kg_sb = consts.tile([128, 1], F32, tag="kgc", name="kgc")
    kyg_sb = consts.tile([128, QH], F32, tag="kygc", name="kygc")
    qb_sb = consts.tile([128, QH], F32, tag="qb", name="qb")
    kb_sb = consts.tile([128, 1], F32, tag="kb", name="kb")
    kyb_sb = consts.tile([128, QH], F32, tag="kyb", name="kyb")
    xm_sb = consts.tile([128, B * NKC0], F32, tag="xm", name="xm")
    ym_sb = consts.tile([128, B * NYKC], F32, tag="ym", name="ym")
    cc_sb = [consts.tile([128, S], DT16, tag=f"cc{b}", name=f"cc{b}")
             for b in range(B)]
    ssp_sb = [consts.tile([128, S], DT16, tag=f"ssp{b}", name=f"ssp{b}")
              for b in range(B)]

    def load_consts():
        nc.gpsimd.dma_start(qg_sb[:, :], t["qgc"].ap())
        nc.gpsimd.dma_start(kg_sb[:, :], t["kgc"].ap())
        nc.gpsimd.dma_start(kyg_sb[:, :], t["kygc"].ap())
        nc.gpsimd.dma_start(qb_sb[:, :], t["qb"].ap())
        nc.gpsimd.dma_start(kb_sb[:, :], t["kb"].ap())
        nc.gpsimd.dma_start(kyb_sb[:, :], t["kyb"].ap())
        nc.gpsimd.dma_start(xm_sb[:, :], t["xmask"].ap())
        nc.gpsimd.dma_start(ym_sb[:, :], t["ymask"].ap())
        for b in range(B):
            nc.gpsimd.dma_start(cc_sb[b][:, :], t["CC"].ap()[b])
            nc.gpsimd.dma_start(ssp_sb[b][:, :], t["SSp"].ap()[b])

    load_consts()

    # ---------------- pools ----------------
    cm_raw = tc.tile_pool(name="p_raw", bufs=1)
    p_raw = cm_raw.__enter__()
    cm_w = tc.tile_pool(name="p_w", bufs=1)
    p_w = cm_w.__enter__()
    cm_x = tc.tile_pool(name="p_x", bufs=2)
    p_x = cm_x.__enter__()
    cm_sq = tc.tile_pool(name="w_sq", bufs=3)
    w_sq = cm_sq.__enter__()
    cm_stat = tc.tile_pool(name="w_stat", bufs=3)
    w_stat = cm_stat.__enter__()

    cm_psA = tc.tile_pool(name="pp_proj", bufs=2, space="PSUM")
    pp_proj = cm_psA.__enter__()
    cm_psV = tc.tile_pool(name="pp_v", bufs=2, space="PSUM")
    pp_v = cm_psV.__enter__()

    xT_r = [t["xT"].ap()[b].rearrange("(c p) s -> p c s", p=128)
            for b in range(B)]

    # first x block + wq strips lead the DMA queue for fast start
    wq_sb = p_w.tile([128, NDC, QW], DT16, tag="wq", name="wq")
    wq_r = t["wq"].ap().rearrange("(c p) m -> p c m", p=128)
    xtb0 = p_x.tile([128, NDC, SB], DT16, tag="xtb", name="xtb")
    for s0, s1 in ((0, 1), (1, 2), (2, 4), (4, 8), (8, 16)):
        nc.sync.dma_start(xtb0[:, s0:s1, :], xT_r[0][:, s0:s1, 0:SB])
        nc.sync.dma_start(wq_sb[:, s0:s1, :], wq_r[:, s0:s1, :])
    del wq_r
    wk_sb = p_w.tile([128, NDC, KW], DT16, tag="wk", name="wk")
    wk_r = t["wk"].ap().rearrange("(c p) m -> p c m", p=128)
    wv_sb = p_w.tile([128, NDC, KW], DT16, tag="wv", name="wv")
    wv_r = t["wv"].ap().rearrange("(c p) m -> p c m", p=128)
    for s0 in range(0, NDC, 8):
        nc.sync.dma_start(wk_sb[:, s0:s0 + 8, :], wk_r[:, s0:s0 + 8, :])
        nc.sync.dma_start(wv_sb[:, s0:s0 + 8, :], wv_r[:, s0:s0 + 8, :])
    yt = [p_w.tile([128, NYC, YL], DT16, tag=f"yt{b}", name=f"yt{b}")
          for b in range(B)]
    wky_sb = p_w.tile([128, NYC, YW], DT16, tag="wky", name="wky")
    wvy_sb = p_w.tile([128, NYC, YW], DT16, tag="wvy", name="wvy")
    nc.gpsimd.dma_start(wky_sb[:, :, :],
                        t["wky"].ap().rearrange("(c p) m -> p c m", p=128))
    nc.gpsimd.dma_start(wvy_sb[:, :, :],
                        t["wvy"].ap().rearrange("(c p) m -> p c m", p=128))
    for bb in range(B):
        nc.gpsimd.dma_start(yt[bb][:, :, :],
                            t["yT"].ap()[bb].rearrange("(c p) s -> p c s",
                                                       p=128))

    qraw = [[p_raw.tile([128, S], DT16, tag=f"qraw{b}{i}",
                        name=f"qraw{b}{i}") for i in range(QH)]
            for b in range(B)]
    kraw = [p_raw.tile([128, S], DT16, tag=f"kraw{b}", name=f"kraw{b}")
            for b in range(B)]
    ykraw = [p_raw.tile([128, QH, YL], DT16, tag=f"ykraw{b}",
                        name=f"ykraw{b}") for b in range(B)]

    cm_qkv = tc.tile_pool(name="p_qkv", bufs=1, side="right")
    p_qkv = cm_qkv.__enter__()
    QT = [[p_qkv.tile([128, S], DT16, tag=f"QT{b}{i}", name=f"QT{b}{i}")
           for i in range(QH)] for b in range(B)]
    KT = [p_qkv.tile([128, S], DT16, tag=f"KT{b}", name=f"KT{b}")
          for b in range(B)]
    vnat = [p_qkv.tile([128, NKC0, 128], DT16, tag=f"vnat{b}",
                       name=f"vnat{b}") for b in range(B)]
    YKT = [p_qkv.tile([128, QH, YL], DT16, tag=f"YKT{b}", name=f"YKT{b}")
           for b in range(B)]
    yvnat = [p_qkv.tile([128, NYKC, YW], DT16, tag=f"yvnat{b}",
                        name=f"yvnat{b}") for b in range(B)]

    cm_rm = tc.tile_pool(name="rows_m", bufs=1, side="right")
    rows_m = cm_rm.__enter__()
    cm_wln = tc.tile_pool(name="w_ln", bufs=1, side="right")
    w_ln = cm_wln.__enter__()
    cm_wln2 = tc.tile_pool(name="w_ln2", bufs=1, side="right")
    w_ln2 = cm_wln2.__enter__()

    def stat_to_row(dram, row, col0, blk, src_f16):
        """partition_all_reduce src [128, blk] f16 -> row0 -> dram row."""
        st = w_stat.tile([128, SB], F32, tag="st", name="st")
        nc.gpsimd.partition_all_reduce(st[:, :blk], src_f16, 128, RED.add)
        nc.gpsimd.dma_start(dram.ap()[row:row + 1, col0:col0 + blk],
                            st[0:1, :blk])

    def proj_batch(b, sbs=range(NSB)):
        for sb in sbs:
            if b == 0 and sb == 0:
                xtb = xtb0
            else:
                xtb = p_x.tile([128, NDC, SB], DT16, tag="xtb", name="xtb")
                for s0 in range(0, NDC, 8):
                    nc.sync.dma_start(
                        xtb[:, s0:s0 + 8, :],
                        xT_r[b][:, s0:s0 + 8, sb * SB:(sb + 1) * SB])
            sl = slice(sb * SB, (sb + 1) * SB)
            # q projections (2 head blocks)
            for i in range(QH):
                ps = pp_proj.tile([128, SB], F32, tag="proj", name="proj")
                for c in range(NDC):
                    nc.tensor.matmul(ps[:, :],
                                     wq_sb[:, c, i * 128:(i + 1) * 128],
                                     xtb[:, c, :], start=(c == 0),
                                     stop=(c == NDC - 1))
                nc.scalar.activation(qraw[b][i][:, sl], ps[:, :], AF.Copy)
            # k projection
            ps = pp_proj.tile([128, SB], F32, tag="proj", name="proj")
            for c in range(NDC):
                nc.tensor.matmul(ps[:, :], wk_sb[:, c, :], xtb[:, c, :],
                                 start=(c == 0), stop=(c == NDC - 1))
            nc.scalar.activation(kraw[b][:, sl], ps[:, :], AF.Copy)
            # v direct [token, hd] layout
            for s4 in range(4):
                ck = sb * 4 + s4
                psv = pp_v.tile([128, KW], F32, tag="pv", name="pv")
                for c in range(NDC):
                    nc.tensor.matmul(
                        psv[:, :], xtb[:, c, s4 * 128:(s4 + 1) * 128],
                        wv_sb[:, c, :], start=(c == 0), stop=(c == NDC - 1))
                nc.scalar.activation(vnat[b][:, ck, :], psv[:, :], AF.Copy)
            # stats: q sum/sumsq, k sum/sumsq (DVE squares, Pool reduce)
            s01 = w_sq.tile([128, SB], DT16, tag="sq", name="sq")
            nc.vector.tensor_tensor(s01[:, :], qraw[b][0][:, sl],
                                    qraw[b][1][:, sl], Alu.add)
            stat_to_row(t["qin"], 2 * b, sb * SB, SB, s01[:, :])
            sq0 = w_sq.tile([128, SB], DT16, tag="sq", name="sq")
            nc.vector.tensor_tensor(sq0[:, :], qraw[b][0][:, sl],
                                    qraw[b][0][:, sl], Alu.mult)
            sq1 = w_sq.tile([128, SB], DT16, tag="sq", name="sq")
            nc.vector.tensor_tensor(sq1[:, :], qraw[b][1][:, sl],
                                    qraw[b][1][:, sl], Alu.mult)
            nc.vector.tensor_tensor(sq0[:, :], sq0[:, :], sq1[:, :],
                                    Alu.add)
            stat_to_row(t["qin"], 2 * b + 1, sb * SB, SB, sq0[:, :])
            stat_to_row(t["kin"], 2 * b, sb * SB, SB, kraw[b][:, sl])
            sqk = w_sq.tile([128, SB], DT16, tag="sq", name="sq")
            nc.vector.tensor_tensor(sqk[:, :], kraw[b][:, sl],
                                    kraw[b][:, sl], Alu.mult)
            stat_to_row(t["kin"], 2 * b + 1, sb * SB, SB, sqk[:, :])

    def proj_y(b):
        for i in range(QH):
            ps = pp_proj.tile([128, SB], F32, tag="proj", name="proj")
            for c in range(NYC):
                nc.tensor.matmul(ps[:, :YL],
                                 wky_sb[:, c, i * 128:(i + 1) * 128],
                                 yt[b][:, c, :], start=(c == 0),
                                 stop=(c == NYC - 1))
            nc.scalar.activation(ykraw[b][:, i, :], ps[:, :YL], AF.Copy)
        for ck in range(NYKC):
            psv = pp_proj.tile([128, SB], F32, tag="proj", name="proj")
            for c in range(NYC):
                nc.tensor.matmul(
                    psv[:, :YW], yt[b][:, c, ck * 128:(ck + 1) * 128],
                    wvy_sb[:, c, :], start=(c == 0), stop=(c == NYC - 1))
            nc.scalar.activation(yvnat[b][:, ck, :], psv[:, :YW], AF.Copy)
        s01 = w_sq.tile([128, SB], DT16, tag="sq", name="sq")
        nc.vector.tensor_tensor(s01[:, :YL], ykraw[b][:, 0, :],
                                ykraw[b][:, 1, :], Alu.add)
        stat_to_row(t["kyin"], 2 * b, 0, YL, s01[:, :YL])
        sq0 = w_sq.tile([128, SB], DT16, tag="sq", name="sq")
        nc.vector.tensor_tensor(sq0[:, :YL], ykraw[b][:, 0, :],
                                ykraw[b][:, 0, :], Alu.mult)
        sq1 = w_sq.tile([128, SB], DT16, tag="sq", name="sq")
        nc.vector.tensor_tensor(sq1[:, :YL], ykraw[b][:, 1, :],
                                ykraw[b][:, 1, :], Alu.mult)
        nc.vector.tensor_tensor(sq0[:, :YL], sq0[:, :YL], sq1[:, :YL],
                                Alu.add)
        stat_to_row(t["kyin"], 2 * b + 1, 0, YL, sq0[:, :YL])

    def all_reduce_batch(b):
        for src, dst in (("qin", "qout"), ("kin", "kout"),
                         ("kyin", "kyout")):
            if t["use_cc"]:
                nc.gpsimd.collective_compute(
                    "AllReduce", Alu.add, replica_groups=t["groups"],
                    ins=[t[src].ap()[2 * b:2 * b + 2].opt()],
                    outs=[t[dst].ap()[2 * b:2 * b + 2].opt()])
            else:
                nc.gpsimd.dma_start(t[dst].ap()[2 * b:2 * b + 2],
                                    t[src].ap()[2 * b:2 * b + 2])

    def moments(src, b, n, inv_scale, eps, length, r_rstd, r_nmr):
        """src rows (2b: sum, 2b+1: sumsq) -> lnr rows r_rstd, r_nmr."""
        J = length // 128

        def rd(row):
            tile_ = rows_m.tile([128, 16], F32, tag=f"m{row % 2}",
                                name=f"m{row % 2}")
            ap = bass.AP(tensor=src.ap().tensor, offset=row * length,
                         ap=[[J, 128], [1, J]])
            nc.sync.dma_start(tile_[:, :J], ap)
            return tile_
        a = rd(2 * b)
        nc.vector.tensor_scalar_mul(a[:, :J], a[:, :J], inv_scale / n)
        bb = rd(2 * b + 1)
        nc.vector.tensor_scalar_mul(bb[:, :J], bb[:, :J], inv_scale / n)
        c = rows_m.tile([128, 16], F32, tag="mc", name="mc")
        nc.vector.tensor_mul(c[:, :J], a[:, :J], a[:, :J])
        nc.vector.tensor_tensor(bb[:, :J], bb[:, :J], c[:, :J],
                                Alu.subtract)
        nc.vector.tensor_scalar_add(bb[:, :J], bb[:, :J], eps)
        # rstd = rsqrt(var+eps), DVE-only (keeps Act on the exp/copy
        # table): seed 0.44 + 0.38/v, then 4 Newton steps
        nc.vector.reciprocal(c[:, :J], bb[:, :J])
        nc.vector.tensor_scalar(out=c[:, :J], in0=c[:, :J],
                                scalar1=0.38, scalar2=0.44,
                                op0=Alu.mult, op1=Alu.add)
        d = rows_m.tile([128, 16], F32, tag="md", name="md")
        for _ in range(4):
            nc.vector.tensor_mul(d[:, :J], c[:, :J], c[:, :J])
            nc.vector.tensor_mul(d[:, :J], d[:, :J], bb[:, :J])
            nc.vector.tensor_scalar(out=d[:, :J], in0=d[:, :J],
                                    scalar1=-0.5, scalar2=1.5,
                                    op0=Alu.mult, op1=Alu.add)
            nc.vector.tensor_mul(c[:, :J], c[:, :J], d[:, :J])
        nc.vector.tensor_mul(a[:, :J], a[:, :J], c[:, :J])
        nc.vector.tensor_scalar_mul(a[:, :J], a[:, :J], -1.0)
        ch = rows_m.tile([128, 16], DT16, tag="mch", name="mch")
        nc.vector.tensor_copy(ch[:, :J], c[:, :J])
        ah = rows_m.tile([128, 16], DT16, tag="mah", name="mah")
        nc.vector.tensor_copy(ah[:, :J], a[:, :J])
        out_r = bass.AP(tensor=t["lnr"].ap().tensor, offset=r_rstd * S,
                        ap=[[J, 128], [1, J]])
        nc.sync.dma_start(out_r, ch[:, :J])
        out_n = bass.AP(tensor=t["lnr"].ap().tensor, offset=r_nmr * S,
                        ap=[[J, 128], [1, J]])
        nc.sync.dma_start(out_n, ah[:, :J])

    def dma_bcast(dst, row, length):
        src_ap = bass.AP(tensor=t["lnr"].ap().tensor, offset=row * S,
                         ap=[[0, 128], [1, length]])
        nc.sync.dma_start(dst[:, :length], src_ap)

    def ln_rope(raw_ap, fin_ap, rg, ng, g_col, b_col, length, rope_b, eng):
        t1 = w_ln2.tile([128, S], DT16, tag="lnt1", name="lnt1")
        eng.tensor_mul(t1[:, :length], raw_ap, rg[:, :length])
        eng.tensor_add(t1[:, :length], t1[:, :length], ng[:, :length])
        nc.vector.tensor_scalar(out=t1[:, :length], in0=t1[:, :length],
                                scalar1=g_col, scalar2=b_col,
                                op0=Alu.mult, op1=Alu.add)
        if rope_b is None:
            nc.vector.tensor_copy(fin_ap, t1[:, :length])
            return
        sw = w_ln2.tile([128, S], DT16, tag="swap", name="swap")
        nc.sync.dma_start(sw[0:64, :length], t1[64:128, :length])
        nc.sync.dma_start(sw[64:128, :length], t1[0:64, :length])
        m1 = w_ln2.tile([128, S], DT16, tag="m1", name="m1")
        nc.vector.tensor_mul(m1[:, :length], t1[:, :length],
                             cc_sb[rope_b][:, :length])
        nc.vector.tensor_mul(sw[:, :length], sw[:, :length],
                             ssp_sb[rope_b][:, :length])
        nc.vector.tensor_add(fin_ap, m1[:, :length], sw[:, :length])

    def moments_batch(b):
        moments(t["qout"], b, H * HD, 1.0, EPS_QK, S, 2 * b, 2 * b + 1)
        moments(t["kyout"], b, KV * HD, 0.5, EPS_KY, YL, 8 + 2 * b,
                9 + 2 * b)
        moments(t["kout"], b, KV * HD, 1.0, EPS_QK, S, 4 + 2 * b,
                5 + 2 * b)

    def lnapply_q(b, eng):
        rg = w_ln.tile([128, S], DT16, tag="bc_rg", name="bc_rg")
        dma_bcast(rg, 2 * b, S)
        ng = w_ln.tile([128, S], DT16, tag="bc_ng", name="bc_ng")
        dma_bcast(ng, 2 * b + 1, S)
        for i in range(QH):
            ln_rope(qraw[b][i][:, :], QT[b][i][:, :], rg, ng,
                    qg_sb[:, i:i + 1], qb_sb[:, i:i + 1], S, b, eng)

    def lnapply_ky(b, eng):
        rg = w_ln.tile([128, S], DT16, tag="bc_rg", name="bc_rg")
        dma_bcast(rg, 8 + 2 * b, YL)
        ng = w_ln.tile([128, S], DT16, tag="bc_ng", name="bc_ng")
        dma_bcast(ng, 9 + 2 * b, YL)
        for i in range(QH):
            ln_rope(ykraw[b][:, i, :], YKT[b][:, i, :], rg, ng,
                    kyg_sb[:, i:i + 1], kyb_sb[:, i:i + 1], YL, None, eng)

    def lnapply_k(b, eng):
        rg = w_ln.tile([128, S], DT16, tag="bc_rg", name="bc_rg")
        dma_bcast(rg, 4 + 2 * b, S)
        ng = w_ln.tile([128, S], DT16, tag="bc_ng", name="bc_ng")
        dma_bcast(ng, 5 + 2 * b, S)
        ln_rope(kraw[b][:, :], KT[b][:, :], rg, ng,
                kg_sb[:, 0:1], kb_sb[:, 0:1], S, b, eng)

    # outY for batch 0 lives in the long-lived right pool: written by the
    # cross-attn groups interleaved into batch-1 projections, read at the
    # batch-0 self-attn tails.
    outY = [[p_qkv.tile([128, S], DT16, tag=f"outY0{h}", name=f"outY0{h}")
             for h in range(QH)], [None, None]]
    outT = [[None, None], [None, None]]
    P = {}
    ncopy = [0]

    def attend(b, qb_i, keys_T, vals, nkc, mask_sb, mask_col0, cross,
               lag=9):
        """Head-paired attention for query block qb_i of batch b.

        PV matmuls lag the score/exp stream by `lag` chunks so the PE
        in-order queue has score work while the previous group's pv PSUM
        bank drains through its denominator chain.
        """
        q0 = qb_i * QB
        lag = min(lag, nkc - 1)
        pv = P["pv"].tile([128, 2 * QB], F32, tag="pv", name="pv")
        acc = P["acc"].tile([128, 2 * QB], DT16, tag="acc", name="acc")
        pts = {}

        def pv_step(c):
            for h in range(QH):
                nc.tensor.matmul(pv[:, h * QB:(h + 1) * QB], vals(h, c),
                                 pts[c][:, h * QB:(h + 1) * QB],
                                 start=(c == 0), stop=(c == nkc - 1))
            del pts[c]

        for c in range(nkc):
            sc = P["sc"].tile([128, 2 * QB], F32, tag="sc", name="sc")
            pt = P["pt"].tile([128, 2 * QB], DT16, tag="ptile",
                              name="ptile")
            pts[c] = pt
            for h in range(QH):
                nc.tensor.matmul(sc[:, h * QB:(h + 1) * QB], keys_T(h, c),
                                 QT[b][h][:, q0:q0 + QB],
                                 start=True, stop=True)
            nc.scalar.activation(
                pt[:, :], sc[:, :], AF.Exp,
                bias=mask_sb[:, mask_col0 + c:mask_col0 + c + 1])
            if c >= lag:
                pv_step(c - lag)
            if c == 0:
                nc.vector.tensor_copy(acc[:, :], pt[:, :])
            else:
                nc.vector.tensor_add(acc[:, :], acc[:, :], pt[:, :])
        for c in range(nkc - lag, nkc):
            pv_step(c)
        den = P["den"].tile([128, 2 * QB], DT16, tag="den", name="den")
        nc.gpsimd.partition_all_reduce(den[:, :], acc[:, :], 128, RED.add)
        rden = P["den"].tile([128, 2 * QB], F32, tag="rden", name="rden")
        nc.vector.reciprocal(rden[:, :], den[:, :])
        for h in range(QH):
            dst = (outY if cross else outT)[b][h][:, q0:q0 + QB]
            nc.vector.tensor_mul(dst, pv[:, h * QB:(h + 1) * QB],
                                 rden[:, h * QB:(h + 1) * QB])
            if not cross:
                nc.vector.tensor_add(dst, dst, outY[b][h][:, q0:q0 + QB])

    def wo_block(b, qb_i):
        last = b == 1 and qb_i == 3
        for st in range(qb_i * 4, qb_i * 4 + 4):
            ob = P["ob"].tile([128, D], DT16, tag="obuf", name="obuf")
            for jc in range(4):
                pso = P["wo"].tile([128, 512], F32, tag="wops",
                                   name="wops")
                for h in range(QH):
                    nc.tensor.matmul(
                        pso[:, :], outT[b][h][:, st * 128:(st + 1) * 128],
                        P["wo_sb"][:, h, jc * 512:(jc + 1) * 512],
                        start=(h == 0), stop=(h == QH - 1))
                on_act = (jc % 2 == 0) if last else \
                    (ncopy[0] % 8 < ((6 if qb_i < 2 else 4) if b == 0 else 2))
                if on_act:
                    nc.scalar.activation(ob[:, jc * 512:(jc + 1) * 512],
                                         pso[:, :], AF.Copy)
                else:
                    nc.vector.tensor_copy(ob[:, jc * 512:(jc + 1) * 512],
                                          pso[:, :])
                ncopy[0] += 1
                if last and jc == 1:
                    nc.sync.dma_start(
                        t["out"].ap()[b][st * 128:(st + 1) * 128, 0:1024],
                        ob[:, 0:1024])
            if last:
                nc.sync.dma_start(
                    t["out"].ap()[b][st * 128:(st + 1) * 128, 1024:D],
                    ob[:, 1024:D])
            else:
                nc.sync.dma_start(
                    t["out"].ap()[b][st * 128:(st + 1) * 128, :], ob[:, :])

    def cross_g(b, qb_i, lag=1):
        attend(b, qb_i,
               lambda h, c, b=b: YKT[b][:, h, c * 128:(c + 1) * 128],
               lambda h, c, b=b: yvnat[b][:, c, h * 128:(h + 1) * 128],
               NYKC, ym_sb, b * NYKC, True, lag=lag)

    def self_g(b, qb_i):
        attend(b, qb_i,
               lambda h, c, b=b: KT[b][:, c * 128:(c + 1) * 128],
               lambda h, c, b=b: vnat[b][:, c, :],
               t["nkc"][b], xm_sb, b * NKC0, False)

    # ============ batch-0 projections ============
    proj_batch(0)
    proj_y(0)
    all_reduce_batch(0)
    moments_batch(0)     # DVE+Act(sqrt): overlaps remaining projections
    lnapply_q(0, nc.vector)
    lnapply_ky(0, nc.vector)
    lnapply_k(0, nc.vector)

    # ===== batch-1 projections with batch-0 cross-attn interleaved =====
    proj_batch(1, [0, 1])
    cm_cpt = tc.tile_pool(name="crs_pt", bufs=2)
    cm_cacc = tc.tile_pool(name="crs_acc", bufs=1)
    cm_cden = tc.tile_pool(name="crs_den", bufs=1)
    cm_csc = tc.tile_pool(name="crs_sc", bufs=1, space="PSUM")
    cm_cpv = tc.tile_pool(name="crs_pv", bufs=1, space="PSUM")
    P.update(pt=cm_cpt.__enter__(), acc=cm_cacc.__enter__(),
             den=cm_cden.__enter__(), sc=cm_csc.__enter__(),
             pv=cm_cpv.__enter__())
    cross_g(0, 0)
    cross_g(0, 1)
    proj_batch(1, [2])
    cross_g(0, 2)
    proj_batch(1, [3])
    cross_g(0, 3)
    proj_y(1)
    all_reduce_batch(1)
    moments_batch(1)     # all Act sqrt done before self-attn exps

    cm_cpv.__exit__(None, None, None)
    cm_csc.__exit__(None, None, None)
    cm_cden.__exit__(None, None, None)
    cm_cacc.__exit__(None, None, None)
    cm_cpt.__exit__(None, None, None)
    cm_psV.__exit__(None, None, None)
    cm_psA.__exit__(None, None, None)
    cm_stat.__exit__(None, None, None)
    cm_sq.__exit__(None, None, None)
    cm_x.__exit__(None, None, None)
    cm_w.__exit__(None, None, None)

    # ============ attention + wo ============
    cm_out = tc.tile_pool(name="p_out", bufs=1)
    p_out = cm_out.__enter__()
    for b in range(B):
        for h in range(QH):
            outT[b][h] = p_out.tile([128, S], DT16, tag=f"outT{b}{h}",
                                    name=f"outT{b}{h}")
    for h in range(QH):
        outY[1][h] = p_out.tile([128, S], DT16, tag=f"outY1{h}",
                                name=f"outY1{h}")
    cm_wo = tc.tile_pool(name="p_wo", bufs=1)
    p_wo = cm_wo.__enter__()
    wo_sb = p_wo.tile([128, QH, D], DT16, tag="wo", name="wo")
    nc.gpsimd.dma_start(wo_sb[:, :, :],
                        t["wo"].ap().rearrange("(c p) m -> p c m", p=128))
    cm_pt = tc.tile_pool(name="w_pt", bufs=10)
    cm_acc = tc.tile_pool(name="w_acc", bufs=2)
    cm_den = tc.tile_pool(name="w_den", bufs=2)
    cm_ob = tc.tile_pool(name="w_ob", bufs=4)
    cm_sc = tc.tile_pool(name="pp_sc", bufs=2, space="PSUM")
    cm_pv = tc.tile_pool(name="pp_pv", bufs=1, space="PSUM")
    cm_po = tc.tile_pool(name="pp_wo", bufs=2, space="PSUM")
    P.update(pt=cm_pt.__enter__(), acc=cm_acc.__enter__(),
             den=cm_den.__enter__(), ob=cm_ob.__enter__(),
             sc=cm_sc.__enter__(), pv=cm_pv.__enter__(),
             wo=cm_po.__enter__(), wo_sb=wo_sb)

    # self-attn with wo one query-block behind as in-order PE filler;
    # batch-1 LN applies (Pool+DVE) slot between batch-0 groups.
    self_g(0, 0)
    lnapply_q(1, nc.vector)
    self_g(0, 1)
    wo_block(0, 0)
    lnapply_ky(1, nc.vector)
    self_g(0, 2)
    wo_block(0, 1)
    lnapply_k(1, nc.vector)
    self_g(0, 3)
    wo_block(0, 2)
    cross_g(1, 0, lag=1)
    self_g(1, 0)
    wo_block(0, 3)
    cross_g(1, 1, lag=1)
    self_g(1, 1)
    wo_block(1, 0)
    cross_g(1, 2, lag=1)
    self_g(1, 2)
    wo_block(1, 1)
    cross_g(1, 3, lag=1)
    self_g(1, 3)
    wo_block(1, 2)
    wo_block(1, 3)

    cm_po.__exit__(None, None, None)
    cm_pv.__exit__(None, None, None)
    cm_sc.__exit__(None, None, None)
    cm_ob.__exit__(None, None, None)
    cm_den.__exit__(None, None, None)
    cm_acc.__exit__(None, None, None)
    cm_pt.__exit__(None, None, None)
    cm_wo.__exit__(None, None, None)
    cm_out.__exit__(None, None, None)
    cm_wln2.__exit__(None, None, None)
    cm_wln.__exit__(None, None, None)
    cm_rm.__exit__(None, None, None)
    cm_qkv.__exit__(None, None, None)
    cm_raw.__exit__(None, None, None)
    cm_consts.__exit__(None, None, None)


def _perm_cols(ncols):
    p = np.arange(ncols).reshape(-1, HD)
    return np.concatenate([p[:, 0::2], p[:, 1::2]], axis=1).reshape(-1)


def _prep_core_inputs(inputs, core):
    c = core
    f32 = np.float32
    x = np.asarray(inputs["x"], f32)
    y = np.asarray(inputs["y"], f32)

    qcols = np.arange(2 * c * HD, (2 * c + 2) * HD)
    kcols = np.arange(c * HD, (c + 1) * HD)
    y0 = ((2 * c) % KV) * HD
    ycols = np.arange(y0, y0 + 2 * HD)
    qperm = qcols[_perm_cols(2 * HD)]
    kperm = kcols[_perm_cols(HD)]
    yperm = ycols[_perm_cols(2 * HD)]

    scale = 1.0 / np.sqrt(HD)
    qg = (np.asarray(inputs["q_norm_g"], f32) * scale)[qperm]
    qb = (np.asarray(inputs["q_norm_b"], f32) * scale)[qperm]
    kg = np.asarray(inputs["k_norm_g"], f32)[kperm]
    kb = np.asarray(inputs["k_norm_b"], f32)[kperm]
    kyg = np.asarray(inputs["ky_norm_g"], f32)[yperm]
    kyb = np.asarray(inputs["ky_norm_b"], f32)[yperm]

    CCm = np.zeros((B, 128, S), f32)
    SSm = np.zeros((B, 128, S), f32)
    for b in range(B):
        cos = np.asarray(inputs["freqs_cos"], f32)[b].T
        sin = np.asarray(inputs["freqs_sin"], f32)[b].T
        CCm[b] = np.concatenate([cos, cos], 0)
        SSm[b] = np.concatenate([-sin, sin], 0)

    xm = np.where(np.asarray(inputs["x_mask"]), 0.0, NEG).astype(f32)
    ym = np.where(np.asarray(inputs["y_mask"]), 0.0, NEG).astype(f32)
    xmt = np.concatenate([xm[b].reshape(NKC0, 128).T for b in range(B)], 1)
    ymt = np.concatenate([ym[b].reshape(NYKC, 128).T for b in range(B)], 1)

    tg = np.tanh(np.asarray(inputs["gate"], f32))
    wvy = np.asarray(inputs["wv_y"], f32)[:, ycols].copy()
    wvy[:, 0:HD] *= tg[2 * c]
    wvy[:, HD:2 * HD] *= tg[2 * c + 1]

    bf = lambda a: np.ascontiguousarray(a).astype(NP16)
    return {
        "xT": bf(np.swapaxes(x, 1, 2)),
        "yT": bf(np.swapaxes(y, 1, 2)),
        "wq": bf(np.asarray(inputs["wq"], f32)[:, qperm]),
        "wk": bf(np.asarray(inputs["wk"], f32)[:, kperm]),
        "wv": bf(np.asarray(inputs["wv"], f32)[:, kcols]),
        "wky": bf(np.asarray(inputs["wk_y"], f32)[:, yperm]),
        "wvy": bf(wvy),
        "wo": bf(np.asarray(inputs["wo"], f32)[qcols, :]),
        "CC": bf(CCm), "SSp": bf(SSm),
        "qgc": np.ascontiguousarray(qg.reshape(QH, HD).T).astype(f32),
        "kgc": np.ascontiguousarray(kg.reshape(1, HD).T).astype(f32),
        "kygc": np.ascontiguousarray(kyg.reshape(QH, HD).T).astype(f32),
        "qb": np.ascontiguousarray(qb.reshape(QH, HD).T).astype(f32),
        "kb": np.ascontiguousarray(kb.reshape(1, HD).T).astype(f32),
        "kyb": np.ascontiguousarray(kyb.reshape(QH, HD).T).astype(f32),
        "xmask": np.ascontiguousarray(xmt).astype(f32),
        "ymask": np.ascontiguousarray(ymt).astype(f32),
    }


def _pick_variant(inputs):
    xm = np.asarray(inputs["x_mask"])
    if not xm[1, 12 * 128:].any():
        return 12
    return NKC0


def _get_runner(nkc1):
    if nkc1 not in _RUNNERS:
        _RUNNERS[nkc1] = _build_program(nkc1)
    return _RUNNERS[nkc1]


def _get_exec(nkc1):
    """Build (once) a cached jitted shard_map executable for the program."""
    if nkc1 not in _EXECS:
        import jax
        from jax.experimental.shard_map import shard_map
        from jax.sharding import Mesh, NamedSharding, PartitionSpec

        nc = _get_runner(nkc1)
        from concourse import bass2jax as b2j
        b2j.install_neuronx_cc_hook()

        pname = (nc.partition_id_tensor.name
                 if nc.partition_id_tensor else None)
        in_names, out_names, out_avals = [], [], []
        for alloc in nc.m.functions[0].allocations:
            if not isinstance(alloc, mybir.MemoryLocationSet):
                continue
            name = alloc.memorylocations[0].name
            if alloc.kind == "ExternalInput":
                if name != pname:
                    in_names.append(name)
            elif alloc.kind == "ExternalOutput":
                out_names.append(name)
                out_avals.append(jax.core.ShapedArray(
                    tuple(alloc.tensor_shape), mybir.dt.np(alloc.dtype)))
        n_params = len(in_names)
        all_in = list(in_names + out_names)
        if pname is not None:
            all_in.append(pname)
        all_in = tuple(all_in)
        donate = tuple(range(n_params, n_params + len(out_names)))

        def _body(*args):
            operands = list(args)
            if pname is not None:
                operands.append(b2j.partition_id_tensor())
            outs = b2j._bass_exec_p.bind(
                *operands, out_avals=tuple(out_avals), in_names=all_in,
                out_names=tuple(out_names),
                lowering_input_output_aliases=(),
                sim_require_finite=True, sim_require_nnan=True, nc=nc)
            return tuple(outs)

        devices = jax.devices()[:N_CORES]
        mesh = Mesh(np.asarray(devices), ("core",))
        nin = n_params + len(out_names)
        sharded = jax.jit(
            shard_map(_body, mesh=mesh,
                      in_specs=(PartitionSpec("core"),) * nin,
                      out_specs=(PartitionSpec("core"),) * len(out_names),
                      check_rep=False),
            donate_argnums=donate, keep_unused=True)
        shd = NamedSharding(mesh, PartitionSpec("core"))
        mk0 = [jax.jit(lambda a=a: __import__("jax.numpy", fromlist=["x"]
                                              ).zeros((N_CORES * a.shape[0],)
                                                      + a.shape[1:], a.dtype),
                       out_shardings=shd) for a in out_avals]
        _EXECS[nkc1] = (sharded, in_names, out_names, out_avals, shd, mk0)
    return _EXECS[nkc1]


def _concat_inputs(in_maps, nkc1):
    sharded, in_names, out_names, out_avals, shd, mk0 = _get_exec(nkc1)
    return [np.concatenate([np.asarray(in_maps[c][nm])
                            for c in range(N_CORES)], axis=0)
            for nm in in_names]


def _exec(concat_in, nkc1, device_put=False):
    import jax
    sharded, in_names, out_names, out_avals, shd, mk0 = _get_exec(nkc1)
    if device_put:
        concat_in = [jax.device_put(a, shd) for a in concat_in]
    outs = sharded(*concat_in, *[f() for f in mk0])
    return dict(zip(out_names, outs))


def kernel(**inputs):
    nkc1 = _pick_variant(inputs)
    in_maps = [_prep_core_inputs(inputs, c) for c in range(N_CORES)]
    outs = _exec(_concat_inputs(in_maps, nkc1), nkc1)
    o = np.asarray(outs["out"]).reshape(N_CORES, B, S, D)
    out = np.zeros((B, S, D), np.float32)
    for c in range(N_CORES):
        out += o[c].astype(np.float32)
    return out
